# revision 17
# baseline (speedup 1.0000x reference)
"""Multi-head attention (B=2, S=2048, D=1024, H=16) on 8 TRN2 NeuronCores.

Sharding: tensor-parallel over heads (2 heads/core).  Each core computes
the qkv projection for its heads (full sequence) and attention, then an
AllToAll redistributes attention outputs so each core holds *all* heads
for a 1/8 slice of the (batch*seq) rows and runs the output projection
locally.  No cross-core reduction needed.

x is pre-transposed AND pre-cast to bf16 on the host (xT [128, D/128,
B*S]) so the device does no activation transpose at all — projections
read xT directly as the matmul moving operand.  Softmax denominators
come from a ones-column appended to V (scores are small, so exp without
max-subtraction is safe); normalization is fused into the AV eviction.

Compute dtype: bf16 matmul operands, fp32 PSUM accumulation.  Score
chunks are 1024 queries wide so each Exp activation instruction covers
[128, 1024], halving Act-engine instruction overhead vs 512-wide.
"""

import sys

sys.path.insert(0, "/opt/trn_rl_repo")

import numpy as np
import ml_dtypes

B, S, D = 2, 2048, 1024
H, HD = 16, 64
NCORES = 8
BS = B * S                 # 4096 flattened rows
HL = H // NCORES           # 2 local heads
CH = HL * HD               # 128 local q/k/v channels
ROWS = BS // NCORES        # 512 output rows per core
P = 128
NDC = D // P               # 8 chunks of the contraction dim D
NKB = S // P               # 16 key blocks per batch
QCW = 1024                 # query-chunk width (one exp instruction per kb)
HD1 = HD + 1               # value channels + ones column

_CACHE = {}


def _interleave(primary, secondary, lead=0):
    """Emit primary tasks in order, spreading secondary tasks between them.
    The first `lead` primary tasks are emitted before any secondary."""
    ns = len(secondary)
    npr = max(len(primary) - lead, 1)
    si = 0
    for i, p in enumerate(primary):
        p()
        tgt = (i + 1 - lead) * ns // npr if i >= lead else 0
        while si < tgt:
            secondary[si]()
            si += 1
    while si < ns:
        secondary[si]()
        si += 1


def _build_program(with_bias: bool, local_a2a: bool = False,
                   repeats: int = 1, loop_n: int = 0, qcw: int = QCW):
    import concourse.bass as bass
    import concourse.mybir as mybir
    import concourse.tile as tile
    from concourse import bacc
    from contextlib import ExitStack

    nqc = S // qcw             # query chunks per batch (2 for qcw=1024)
    nbb = HL * nqc             # blocks per batch (4)
    nqt = qcw // P             # AV row-tiles per block (8)
    dt = mybir.dt
    AF = mybir.ActivationFunctionType
    bf, f32 = dt.bfloat16, dt.float32

    nc = bacc.Bacc()

    xT_d = nc.dram_tensor("xT", [P, NDC, BS], bf, kind="ExternalInput")
    wq = nc.dram_tensor("wq", [P, NDC, CH], bf, kind="ExternalInput")
    wk = nc.dram_tensor("wk", [P, NDC, CH], bf, kind="ExternalInput")
    wv = nc.dram_tensor("wv", [P, NDC, CH], bf, kind="ExternalInput")
    wo = nc.dram_tensor("wo", [P, NCORES, D], bf, kind="ExternalInput")
    if with_bias:
        bq = nc.dram_tensor("bq", [1, CH], bf, kind="ExternalInput")
        bk = nc.dram_tensor("bk", [1, CH], bf, kind="ExternalInput")
        bv = nc.dram_tensor("bv", [1, CH], bf, kind="ExternalInput")
        ob = nc.dram_tensor("ob", [1, D], bf, kind="ExternalInput")
    y = nc.dram_tensor("y", [ROWS, D], f32, kind="ExternalOutput")

    with tile.TileContext(nc) as tc, ExitStack() as ctx:
        const = ctx.enter_context(tc.tile_pool(name="const", bufs=1))
        wq_sb = const.tile([P, NDC, CH], bf)
        wk_sb = const.tile([P, NDC, CH], bf)
        wv_sb = const.tile([P, NDC, CH], bf)
        wo_sb = const.tile([P, NCORES, D], bf)
        if with_bias:
            bq_sb = const.tile([1, CH], bf)
            bk_sb = const.tile([1, CH], bf)
            bv_sb = const.tile([1, CH], bf)
            ob_sb = const.tile([1, D], bf)
            ones_row = const.tile([1, 512], bf)

        big = ctx.enter_context(tc.tile_pool(name="big", bufs=1))
        xT = big.tile([P, NDC, BS], bf)                     # [d%128, d//128, row]
        qT = big.tile([P, BS], bf)                          # q channel-major
        kT = big.tile([P, BS], bf)                          # k channel-major
        v_aug = big.tile([P, B * NKB, HL * HD1], bf)        # v row-major + ones
        valsT = big.tile([P, BS], bf)                       # attn out, ch-major
        vfull = big.tile([P, NCORES, ROWS], bf)             # gathered, for oproj
        # exp ring: AV matmuls trail the exps by AV_LAG key-blocks, so only
        # a short rotation of kb slices is live at once
        ET_DEPTH = 6
        et6 = big.tile([P, ET_DEPTH, qcw], bf)

        small = ctx.enter_context(tc.tile_pool(name="small", bufs=4))
        smallb = ctx.enter_context(tc.tile_pool(name="smallb", bufs=2))
        outp = ctx.enter_context(tc.tile_pool(name="outp", bufs=2))

        # PSUM budget (8 banks): pscore 2 bufs x 2 banks (1024-wide f32),
        # pbig 2 x 1 (projections), paT 1 x 2 (transposed-AV accumulators)
        pbig = ctx.enter_context(tc.tile_pool(name="pbig", bufs=2, space="PSUM"))
        pscore = ctx.enter_context(tc.tile_pool(name="pscore", bufs=2, space="PSUM"))
        pav = ctx.enter_context(tc.tile_pool(name="pav", bufs=1, space="PSUM"))
        NG = qcw // 512
        paT = pav.tile([P, NG, 512], f32)

        dram = ctx.enter_context(tc.tile_pool(name="dram", bufs=1, space="DRAM"))
        # the AllToAll is split into two half-payload collectives (head 0 /
        # head 1 channel halves) so the first can run under live attention
        ccA_in = dram.tile([NCORES, HD, ROWS], bf)
        ccA_out = dram.tile([NCORES, HD, ROWS], bf)
        ccB_in = dram.tile([NCORES, HD, ROWS], bf)
        ccB_out = dram.tile([NCORES, HD, ROWS], bf)

        # ones columns for the softmax-denominator trick; value columns are
        # overwritten by the v-projection evictions
        for h in range(HL):
            nc.vector.memset(v_aug[:, :, h * HD1 + HD], 1.0)

        # ---------------- task builders ----------------

        def t_wload(wsb, wdram):
            return lambda: nc.sync.dma_start(out=wsb[:], in_=wdram[:])

        def t_bias_loads():
            def go():
                nc.sync.dma_start(out=bq_sb[:], in_=bq[:])
                nc.sync.dma_start(out=bk_sb[:], in_=bk[:])
                nc.sync.dma_start(out=bv_sb[:], in_=bv[:])
                nc.sync.dma_start(out=ob_sb[:], in_=ob[:])
                nc.vector.memset(ones_row[:], 1.0)
            return go

        def t_xload(b, rc):
            # one 512-column chunk of xT, all depth slices
            def go():
                c0 = b * S + rc * 512
                nc.sync.dma_start(
                    out=xT[:, :, c0:c0 + 512], in_=xT_d[:, :, c0:c0 + 512]
                )
            return go

        def t_vproj(st):
            def go():
                pv = pbig.tile([P, CH], f32, tag="pk", name="pv")
                for c in range(NDC):
                    nc.tensor.matmul(
                        pv[:],
                        lhsT=xT[:, c, st * P:(st + 1) * P],
                        rhs=wv_sb[:, c, :],
                        start=(c == 0),
                        stop=(c == NDC - 1 and not with_bias),
                    )
                if with_bias:
                    nc.tensor.matmul(
                        pv[:], lhsT=ones_row[:, 0:P], rhs=bv_sb[:],
                        start=False, stop=True,
                    )
                for h in range(HL):
                    nc.vector.tensor_copy(
                        out=v_aug[:, st, h * HD1:h * HD1 + HD],
                        in_=pv[:, h * HD:(h + 1) * HD],
                    )
            return go

        def t_kqproj(b, which, qc):
            # qc indexes 512-wide column chunks (0..3 per batch)
            def go():
                wsb, dst = (wk_sb, kT) if which == "k" else (wq_sb, qT)
                base = b * S + qc * 512
                pq = pbig.tile([P, 512], f32, tag="pk", name="pq")
                for c in range(NDC):
                    nc.tensor.matmul(
                        pq[:],
                        lhsT=wsb[:, c, :],
                        rhs=xT[:, c, base:base + 512],
                        start=(c == 0),
                        stop=(c == NDC - 1 and not with_bias),
                    )
                if with_bias:
                    nc.tensor.matmul(
                        pq[:],
                        lhsT=(bk_sb if which == "k" else bq_sb)[:],
                        rhs=ones_row[:],
                        start=False, stop=True,
                    )
                nc.vector.tensor_copy(out=dst[:, base:base + 512], in_=pq[:])
            return go

        # attention blocks, transposed-AV form: per (b, h, qc) block and
        # key-block kb there is one score task (2 matmuls + a 1024-wide
        # exp) and one AV task (one 512-col matmul per query group,
        # accumulating [65ch x 512q] into paT; partition 64 collects the
        # softmax denominator via the ones column in v_aug).  AV tasks
        # trail the score stream by AV_LAG key-blocks (the emission queue
        # in emit_body), and each block ends with one chain task per group:
        # reciprocal of the denominator row, a partition broadcast of it
        # (gpsimd), and a single fused normalize-multiply into valsT.
        def score_task(b, h, qc, kb, t):
            hp = h * HD
            qbase = b * S + qc * qcw

            def go():
                kbase = b * S + kb * P
                ps = pscore.tile([P, qcw], f32, tag="ps", name="ps")
                for qh in range(qcw // 512):
                    nc.tensor.matmul(
                        ps[:, qh * 512:(qh + 1) * 512],
                        lhsT=kT[hp:hp + HD, kbase:kbase + P],
                        rhs=qT[hp:hp + HD,
                               qbase + qh * 512:qbase + (qh + 1) * 512],
                        start=True,
                        stop=True,
                    )
                nc.scalar.activation(
                    et6[:, t % ET_DEPTH, :], ps[:], AF.Exp, scale=0.125
                )
            return go

        def avmm_task(b, h, kb, t):
            def go():
                for g in range(NG):
                    nc.tensor.matmul(
                        paT[0:HD1, g, :],
                        lhsT=v_aug[:, b * NKB + kb, h * HD1:(h + 1) * HD1],
                        rhs=et6[:, t % ET_DEPTH, g * 512:(g + 1) * 512],
                        start=(kb == 0),
                        stop=(kb == NKB - 1),
                        skip_group_check=True,
                    )
            return go

        def chain_task(b, h, qc, g):
            hp = h * HD

            def go():
                rcg = small.tile([1, 512], f32, tag="rc", name="rc")
                nc.vector.reciprocal(rcg[:], paT[HD:HD1, g, :])
                rbg = smallb.tile([HD, 512], f32, tag="rb", name="rb")
                nc.gpsimd.partition_broadcast(rbg[:], rcg[:])
                col = b * S + qc * qcw + g * 512
                nc.vector.tensor_mul(
                    out=valsT[hp:hp + HD, col:col + 512],
                    in0=paT[0:HD, g, :],
                    in1=rbg[:],
                )
            return go

        def t_ccdma(half, j):
            ccin = ccA_in if half == 0 else ccB_in
            hp = half * HD
            return lambda: nc.sync.dma_start(
                out=ccin[j], in_=valsT[hp:hp + HD, j * ROWS:(j + 1) * ROWS]
            )

        def t_a2a(half):
            ccin, ccout = (ccA_in, ccA_out) if half == 0 else (ccB_in, ccB_out)

            def go():
                if local_a2a:
                    # stand-in for the real AllToAll in loop-timing builds;
                    # p-leading layout prices it like the real collective
                    # (~3.2us vs ~3.4us measured for a 0.5MB AllToAll)
                    nc.sync.dma_start(
                        out=ccout.rearrange("i p r -> p i r"),
                        in_=ccin.rearrange("i p r -> p i r"),
                    )
                else:
                    nc.gpsimd.collective_compute(
                        "AllToAll",
                        mybir.AluOpType.bypass,
                        replica_groups=[list(range(NCORES))],
                        ins=[ccin[:]],
                        outs=[ccout[:]],
                    )
            return go

        def t_vfull(half):
            ccout = ccA_out if half == 0 else ccB_out
            hp = half * HD
            return lambda: nc.sync.dma_start(
                out=vfull[hp:hp + HD, :, :],
                in_=ccout.rearrange("i p r -> p i r"),
            )

        def t_oproj(rt, dh):
            def go():
                po = pscore.tile([P, 512], f32, tag="ps", name="po")
                for c in range(NCORES):
                    nc.tensor.matmul(
                        po[:],
                        lhsT=vfull[:, c, rt * P:(rt + 1) * P],
                        rhs=wo_sb[:, c, dh * 512:(dh + 1) * 512],
                        start=(c == 0),
                        stop=(c == NCORES - 1 and not with_bias),
                    )
                if with_bias:
                    nc.tensor.matmul(
                        po[:], lhsT=ones_row[:, 0:P],
                        rhs=ob_sb[:, dh * 512:(dh + 1) * 512],
                        start=False, stop=True,
                    )
                osb = outp.tile([P, 512], f32, tag="osb", name="osb")
                nc.vector.tensor_copy(out=osb[:], in_=po[:])
                nc.sync.dma_start(
                    out=y[rt * P:(rt + 1) * P, dh * 512:(dh + 1) * 512],
                    in_=osb[:],
                )
            return go

        # ---------------- emission (software pipeline) ----------------
        def emit_body(load_weights):
            # front: first x chunk + k/q projections for the first block's
            # initial key blocks and query columns.  DMA queue order matters:
            # the first score matmul waits on x chunk 0 + wk + wq, so those
            # go first; wv and the remaining x chunks follow.
            t_xload(0, 0)()
            if load_weights:
                t_wload(wk_sb, wk)()
                t_wload(wq_sb, wq)()
            t_xload(0, 1)()
            if load_weights:
                t_wload(wv_sb, wv)()
                if with_bias:
                    t_bias_loads()()
            t_kqproj(0, "k", 0)()
            t_kqproj(0, "q", 0)()
            if qcw > 512:
                t_kqproj(0, "q", 1)()
            t_xload(0, 2)()
            t_xload(0, 3)()

            block_ids = [(b, h, qc) for b in range(B) for h in range(HL)
                         for qc in range(nqc)]
            nblk = len(block_ids)

            from collections import defaultdict
            extras = defaultdict(list)       # before the prev block's AVs
            extras_late = defaultdict(list)  # after the prev block's AVs
            tail_tasks = []
            weave = defaultdict(list)        # woven into score tasks: (pos, task)

            def sched(idx, task, late=True):
                if idx < nblk:
                    (extras_late if late else extras)[idx].append(task)
                else:
                    tail_tasks.append(task)

            # cc DMAs: slot j's data comes from block blkX's AV tasks, which
            # run as part of block blkX+1's mix -> schedule the DMA late in
            # block blkX+1 (after those AVs).
            lastA = 0
            for b in range(B):
                for q in range(4):
                    j = b * 4 + q
                    qc_of = q * 512 // qcw
                    blkA = b * nbb + qc_of
                    blkB = b * nbb + nqc + qc_of
                    sched(blkA + 1, t_ccdma(0, j))
                    sched(blkB + 1, t_ccdma(1, j))
                    lastA = max(lastA, blkA + 1)
            sched(lastA, t_a2a(0))
            sched(lastA, t_vfull(0))
            tail_tasks += [t_a2a(1), t_vfull(1)]
            if load_weights:
                sched(nbb, t_wload(wo_sb, wo))

            if nqc == 2:
                # hand schedule for qcw=1024: batch-1 prep deferred into the
                # Act-bound later blocks of batch 0 so the PE-heavy prep
                # doesn't starve the Activation engine early on.
                sched(1, t_xload(1, 0), late=False)
                sched(1, t_xload(1, 1), late=False)
                sched(2, t_xload(1, 2), late=False)
                sched(2, t_xload(1, 3), late=False)
                for t in (t_kqproj(1, "k", 0), t_kqproj(1, "q", 0),
                          t_kqproj(1, "q", 1)):
                    sched(3, t, late=False)
                for st in range(NKB, NKB + 8):
                    sched(3, t_vproj(st))
                # woven into block 4 (b1,h0,qc0): k chunk c before score
                # kb 4c, remaining b1 v tiles spread along
                weave[4] = [
                    (4, t_kqproj(1, "k", 1)), (5, t_vproj(NKB + 8)),
                    (6, t_vproj(NKB + 9)), (7, t_vproj(NKB + 10)),
                    (8, t_kqproj(1, "k", 2)), (9, t_vproj(NKB + 11)),
                    (10, t_vproj(NKB + 12)), (11, t_vproj(NKB + 13)),
                    (12, t_kqproj(1, "k", 3)), (13, t_vproj(NKB + 14)),
                    (14, t_vproj(NKB + 15)),
                ]
                sched(4, t_kqproj(1, "q", 2))
                sched(4, t_kqproj(1, "q", 3))
            else:
                # generic fallback (qcw=512)
                for b in range(B):
                    for c in range(4):
                        first_qc = c * 512 // qcw
                        if b == 0 and first_qc == 0:
                            continue
                        if b == 0:
                            extras[first_qc - 1].insert(0, t_kqproj(b, "q", c))
                b1_tasks = []
                for rc in range(4):
                    b1_tasks.append(t_xload(1, rc))
                for rc in range(4):
                    b1_tasks.append(t_kqproj(1, "k", rc))
                    b1_tasks.append(t_kqproj(1, "q", rc))
                for st in range(NKB, 2 * NKB):
                    b1_tasks.append(t_vproj(st))
                nspread = nbb - 1
                for i, task in enumerate(b1_tasks):
                    sched(1 + i * nspread // len(b1_tasks), task, late=False)

            AV_LAG = 4
            av_queue = []

            def mk_unit(sgo, ago):
                def go():
                    sgo()
                    av_queue.append(ago)
                    while len(av_queue) > AV_LAG:
                        av_queue.pop(0)()
                return go

            for i, (b, h, qc) in enumerate(block_ids):
                u = [mk_unit(score_task(b, h, qc, kb, i * NKB + kb),
                             avmm_task(b, h, kb, i * NKB + kb))
                     for kb in range(NKB)]
                if i == 0:
                    # weave k chunks 1-3, v tiles 0..15 and (for qcw=1024)
                    # q chunks 2-3 under block 0.  Constraints: score kb
                    # 4c..4c+3 needs k chunk c; the AV matmul for kb (which
                    # trails by AV_LAG units) needs v tile kb; block 1
                    # needs q chunks 2-3.
                    kp = [t_kqproj(0, "k", c) for c in (1, 2, 3)]
                    vp = [t_vproj(st) for st in range(NKB)]
                    qx = ([t_kqproj(0, "q", 2), t_kqproj(0, "q", 3)]
                          if nqc == 2 else [])
                    primary = (u[0:2] + vp[0:3]
                               + u[2:3] + vp[3:4] + u[3:4] + kp[0:1] + vp[4:5]
                               + u[4:5] + vp[5:6] + u[5:6] + vp[6:7]
                               + u[6:7] + vp[7:8] + u[7:8] + kp[1:2] + vp[8:9]
                               + u[8:9] + vp[9:10] + u[9:10] + vp[10:11]
                               + u[10:11] + vp[11:12] + u[11:12] + kp[2:3]
                               + vp[12:13] + u[12:13] + qx[0:1] + vp[13:14]
                               + u[13:14] + vp[14:16] + qx[1:2] + u[14:16])
                elif weave.get(i):
                    primary = []
                    wv_ = sorted(weave[i], key=lambda pt_: pt_[0])
                    wi = 0
                    for si, task in enumerate(u):
                        while wi < len(wv_) and wv_[wi][0] <= si:
                            primary.append(wv_[wi][1])
                            wi += 1
                        primary.append(task)
                    primary += [t for _, t in wv_[wi:]]
                else:
                    primary = u
                mix = extras.get(i, [])[:] + extras_late.get(i, [])
                _interleave(primary, mix, lead=2)
                for g in range(NG):
                    av_queue.append(chain_task(b, h, qc, g))
            while av_queue:
                av_queue.pop(0)()
            for task in tail_tasks:
                task()

            # ---- output projection ----
            for rt in range(ROWS // P):
                for dh in range(D // 512):
                    t_oproj(rt, dh)()

        if loop_n > 1:
            t_wload(wv_sb, wv)()
            t_wload(wk_sb, wk)()
            t_wload(wq_sb, wq)()
            t_wload(wo_sb, wo)()
            if with_bias:
                t_bias_loads()()
            with tc.For_i(0, loop_n, 1):
                emit_body(load_weights=False)
        else:
            for rep in range(repeats):
                emit_body(load_weights=(rep == 0))

    nc.compile()
    return nc


def get_program(with_bias: bool, local_a2a: bool = False,
                repeats: int = 1, loop_n: int = 0, qcw: int = QCW, **_ignored):
    key = (with_bias, local_a2a, repeats, loop_n, qcw)
    if key not in _CACHE:
        _CACHE[key] = _build_program(with_bias, local_a2a, repeats, loop_n, qcw)
    return _CACHE[key]


def make_in_maps(x, qkv_w, qkv_b, o_w, o_b):
    """Host-side sharding: slice per-head weight rows, transpose x and the
    weights to the layouts the kernel consumes, cast to bf16."""
    bfnp = ml_dtypes.bfloat16
    x2 = np.asarray(x, np.float32).reshape(BS, D)
    # xT [128, NDC, BS]: element (p, c, r) = x[r, c*128 + p]
    xT = np.ascontiguousarray(
        x2.T.reshape(NDC, P, BS).transpose(1, 0, 2).astype(bfnp)
    )

    qkv_w = np.asarray(qkv_w, np.float32)
    o_w = np.asarray(o_w, np.float32)
    qkv_b = np.asarray(qkv_b, np.float32)
    o_b = np.asarray(o_b, np.float32)

    with_bias = bool(np.any(qkv_b) or np.any(o_b))

    woT = np.ascontiguousarray(
        o_w.T.reshape(NCORES, P, D).transpose(1, 0, 2).astype(bfnp)
    )
    ob_host = np.ascontiguousarray(o_b.reshape(1, D).astype(bfnp))

    in_maps = []
    for m in range(NCORES):
        heads = [m * HL + h for h in range(HL)]
        q_rows = np.concatenate([qkv_w[h * 3 * HD:h * 3 * HD + HD] for h in heads])
        k_rows = np.concatenate(
            [qkv_w[h * 3 * HD + HD:h * 3 * HD + 2 * HD] for h in heads]
        )
        v_rows = np.concatenate(
            [qkv_w[h * 3 * HD + 2 * HD:h * 3 * HD + 3 * HD] for h in heads]
        )

        def wt(rows):
            # [CH, D] -> [D, CH] -> [p, chunk, CH]
            return np.ascontiguousarray(
                rows.T.reshape(NDC, P, CH).transpose(1, 0, 2).astype(bfnp)
            )

        im = {
            "xT": xT,
            "wq": wt(q_rows),
            "wk": wt(k_rows),
            "wv": wt(v_rows),
            "wo": woT,
        }
        if with_bias:
            bqv = np.concatenate(
                [qkv_b[h * 3 * HD:h * 3 * HD + HD] for h in heads]
            )
            bkv = np.concatenate(
                [qkv_b[h * 3 * HD + HD:h * 3 * HD + 2 * HD] for h in heads]
            )
            bvv = np.concatenate(
                [qkv_b[h * 3 * HD + 2 * HD:h * 3 * HD + 3 * HD] for h in heads]
            )
            im["bq"] = np.ascontiguousarray(bqv.reshape(1, CH).astype(bfnp))
            im["bk"] = np.ascontiguousarray(bkv.reshape(1, CH).astype(bfnp))
            im["bv"] = np.ascontiguousarray(bvv.reshape(1, CH).astype(bfnp))
            im["ob"] = ob_host
        in_maps.append(im)
    return in_maps, with_bias


def kernel(x, qkv_w, qkv_b, o_w, o_b):
    from concourse.bass_utils import run_bass_kernel_spmd

    in_maps, with_bias = make_in_maps(x, qkv_w, qkv_b, o_w, o_b)
    nc = get_program(with_bias)
    res = run_bass_kernel_spmd(nc, in_maps, list(range(NCORES)))
    out = np.concatenate([res.results[m]["y"] for m in range(NCORES)], axis=0)
    return np.ascontiguousarray(out.reshape(B, S, D))


# revision 20
# speedup vs baseline: 1.1783x; 1.1783x over previous
"""Multi-head attention (B=2, S=2048, D=1024, H=16) on 8 TRN2 NeuronCores.

Sharding: tensor-parallel over heads (2 heads/core).  Each core computes
the qkv projection for its heads (full sequence) and attention, then an
AllToAll redistributes attention outputs so each core holds *all* heads
for a 1/8 slice of the (batch*seq) rows and runs the output projection
locally.  No cross-core reduction needed.

x is pre-transposed AND pre-cast to bf16 on the host (xT [128, D/128,
B*S]) so the device does no activation transpose at all — projections
read xT directly as the matmul moving operand.  Softmax denominators
come from a ones-column appended to V (scores are small, so exp without
max-subtraction is safe); normalization is fused into the AV eviction.

Compute dtype: bf16 matmul operands, fp32 PSUM accumulation.  Score
chunks are 1024 queries wide so each Exp activation instruction covers
[128, 1024], halving Act-engine instruction overhead vs 512-wide.
"""

import sys

sys.path.insert(0, "/opt/trn_rl_repo")

import numpy as np
import ml_dtypes

B, S, D = 2, 2048, 1024
H, HD = 16, 64
NCORES = 8
BS = B * S                 # 4096 flattened rows
HL = H // NCORES           # 2 local heads
CH = HL * HD               # 128 local q/k/v channels
ROWS = BS // NCORES        # 512 output rows per core
P = 128
NDC = D // P               # 8 chunks of the contraction dim D
NKB = S // P               # 16 key blocks per batch
QCW = 1024                 # query-chunk width (one exp instruction per kb)
HD1 = HD + 1               # value channels + ones column

_CACHE = {}


def _interleave(primary, secondary, lead=0):
    """Emit primary tasks in order, spreading secondary tasks between them.
    The first `lead` primary tasks are emitted before any secondary."""
    ns = len(secondary)
    npr = max(len(primary) - lead, 1)
    si = 0
    for i, p in enumerate(primary):
        p()
        tgt = (i + 1 - lead) * ns // npr if i >= lead else 0
        while si < tgt:
            secondary[si]()
            si += 1
    while si < ns:
        secondary[si]()
        si += 1


def _build_program(with_bias: bool, local_a2a: bool = False,
                   repeats: int = 1, loop_n: int = 0, qcw: int = QCW):
    import concourse.bass as bass
    import concourse.mybir as mybir
    import concourse.tile as tile
    from concourse import bacc
    from concourse.masks import make_identity
    from contextlib import ExitStack

    nqc = S // qcw             # query chunks per batch (2 for qcw=1024)
    nbb = HL * nqc             # blocks per batch (4)
    nqt = qcw // P             # AV row-tiles per block (8)
    dt = mybir.dt
    AF = mybir.ActivationFunctionType
    bf, f32 = dt.bfloat16, dt.float32

    nc = bacc.Bacc()

    xT_d = nc.dram_tensor("xT", [P, NDC, BS], bf, kind="ExternalInput")
    wq = nc.dram_tensor("wq", [P, NDC, CH], bf, kind="ExternalInput")
    wk = nc.dram_tensor("wk", [P, NDC, CH], bf, kind="ExternalInput")
    wv = nc.dram_tensor("wv", [P, NDC, CH], bf, kind="ExternalInput")
    wo = nc.dram_tensor("wo", [P, NCORES, D], bf, kind="ExternalInput")
    if with_bias:
        bq = nc.dram_tensor("bq", [1, CH], bf, kind="ExternalInput")
        bk = nc.dram_tensor("bk", [1, CH], bf, kind="ExternalInput")
        bv = nc.dram_tensor("bv", [1, CH], bf, kind="ExternalInput")
        ob = nc.dram_tensor("ob", [1, D], bf, kind="ExternalInput")
    y = nc.dram_tensor("y", [ROWS, D], f32, kind="ExternalOutput")

    with tile.TileContext(nc) as tc, ExitStack() as ctx:
        const = ctx.enter_context(tc.tile_pool(name="const", bufs=1))
        ident = const.tile([P, P], bf)
        make_identity(nc, ident[:])

        wq_sb = const.tile([P, NDC, CH], bf)
        wk_sb = const.tile([P, NDC, CH], bf)
        wv_sb = const.tile([P, NDC, CH], bf)
        wo_sb = const.tile([P, NCORES, D], bf)
        if with_bias:
            bq_sb = const.tile([1, CH], bf)
            bk_sb = const.tile([1, CH], bf)
            bv_sb = const.tile([1, CH], bf)
            ob_sb = const.tile([1, D], bf)
            ones_row = const.tile([1, 512], bf)

        big = ctx.enter_context(tc.tile_pool(name="big", bufs=1))
        xT = big.tile([P, NDC, BS], bf)                     # [d%128, d//128, row]
        qT = big.tile([P, BS], bf)                          # q channel-major
        kT = big.tile([P, BS], bf)                          # k channel-major
        v_aug = big.tile([P, B * NKB, HL * HD1], bf)        # v row-major + ones
        valsT = big.tile([P, BS], bf)                       # attn out, ch-major
        vfull = big.tile([P, NCORES, ROWS], bf)             # gathered, for oproj
        expp = ctx.enter_context(tc.tile_pool(name="expp", bufs=2))
        small = ctx.enter_context(tc.tile_pool(name="small", bufs=4))
        outp = ctx.enter_context(tc.tile_pool(name="outp", bufs=2))

        # PSUM budget (8 banks): pscore 2 bufs x 2 banks (1024-wide f32),
        # pbig 2 x 1 (projections), pav 1, pt 1 (AV transpose staging)
        pt = ctx.enter_context(tc.tile_pool(name="pt", bufs=1, space="PSUM"))
        pbig = ctx.enter_context(tc.tile_pool(name="pbig", bufs=2, space="PSUM"))
        pscore = ctx.enter_context(tc.tile_pool(name="pscore", bufs=2, space="PSUM"))
        pav = ctx.enter_context(tc.tile_pool(name="pav", bufs=1, space="PSUM"))

        dram = ctx.enter_context(tc.tile_pool(name="dram", bufs=1, space="DRAM"))
        # the AllToAll is split into two half-payload collectives (head 0 /
        # head 1 channel halves) so the first can run under live attention
        ccA_in = dram.tile([NCORES, HD, ROWS], bf)
        ccA_out = dram.tile([NCORES, HD, ROWS], bf)
        ccB_in = dram.tile([NCORES, HD, ROWS], bf)
        ccB_out = dram.tile([NCORES, HD, ROWS], bf)

        # ones columns for the softmax-denominator trick; value columns are
        # overwritten by the v-projection evictions
        for h in range(HL):
            nc.vector.memset(v_aug[:, :, h * HD1 + HD], 1.0)

        # ---------------- task builders ----------------

        def t_wload(wsb, wdram):
            return lambda: nc.sync.dma_start(out=wsb[:], in_=wdram[:])

        def t_bias_loads():
            def go():
                nc.sync.dma_start(out=bq_sb[:], in_=bq[:])
                nc.sync.dma_start(out=bk_sb[:], in_=bk[:])
                nc.sync.dma_start(out=bv_sb[:], in_=bv[:])
                nc.sync.dma_start(out=ob_sb[:], in_=ob[:])
                nc.vector.memset(ones_row[:], 1.0)
            return go

        def t_xload(b, rc):
            # one 512-column chunk of xT, all depth slices
            def go():
                c0 = b * S + rc * 512
                nc.sync.dma_start(
                    out=xT[:, :, c0:c0 + 512], in_=xT_d[:, :, c0:c0 + 512]
                )
            return go

        def t_vproj(st):
            def go():
                pv = pbig.tile([P, CH], f32, tag="pk", name="pv")
                for c in range(NDC):
                    nc.tensor.matmul(
                        pv[:],
                        lhsT=xT[:, c, st * P:(st + 1) * P],
                        rhs=wv_sb[:, c, :],
                        start=(c == 0),
                        stop=(c == NDC - 1 and not with_bias),
                    )
                if with_bias:
                    nc.tensor.matmul(
                        pv[:], lhsT=ones_row[:, 0:P], rhs=bv_sb[:],
                        start=False, stop=True,
                    )
                for h in range(HL):
                    nc.vector.tensor_copy(
                        out=v_aug[:, st, h * HD1:h * HD1 + HD],
                        in_=pv[:, h * HD:(h + 1) * HD],
                    )
            return go

        def t_kqproj(b, which, qc):
            # qc indexes 512-wide column chunks (0..3 per batch)
            def go():
                wsb, dst = (wk_sb, kT) if which == "k" else (wq_sb, qT)
                base = b * S + qc * 512
                pq = pbig.tile([P, 512], f32, tag="pk", name="pq")
                for c in range(NDC):
                    nc.tensor.matmul(
                        pq[:],
                        lhsT=wsb[:, c, :],
                        rhs=xT[:, c, base:base + 512],
                        start=(c == 0),
                        stop=(c == NDC - 1 and not with_bias),
                    )
                if with_bias:
                    nc.tensor.matmul(
                        pq[:],
                        lhsT=(bk_sb if which == "k" else bq_sb)[:],
                        rhs=ones_row[:],
                        start=False, stop=True,
                    )
                nc.vector.tensor_copy(out=dst[:, base:base + 512], in_=pq[:])
            return go

        # attention blocks: per (b, h, qc) -> score tasks (one per kb) and
        # AV tasks (one per 128-query tile)
        def score_tasks(b, h, qc, et):
            hp = h * HD
            qbase = b * S + qc * qcw
            tasks = []

            def mk(kb):
                def go():
                    kbase = b * S + kb * P
                    ps = pscore.tile([P, qcw], f32, tag="ps", name="ps")
                    for qh in range(qcw // 512):
                        nc.tensor.matmul(
                            ps[:, qh * 512:(qh + 1) * 512],
                            lhsT=kT[hp:hp + HD, kbase:kbase + P],
                            rhs=qT[hp:hp + HD,
                                   qbase + qh * 512:qbase + (qh + 1) * 512],
                            start=True,
                            stop=True,
                        )
                    nc.scalar.activation(et[:, kb, :], ps[:], AF.Exp, scale=0.125)
                return go

            for kb in range(NKB):
                tasks.append(mk(kb))
            return tasks

        def av_tasks(b, h, qc, et):
            """Returns interleaved [mm0, mm1, ev0, mm2, ev1, ...]: the PE
            transpose in ev_k waits on a DVE chain, so it is emitted one
            AV-tile later than its matmuls to hide the cross-engine
            latency.  4 sub-slices inside the single pav/pt banks keep the
            tiles independent."""
            hp = h * HD
            qbase = b * S + qc * qcw
            pa2 = pav.tile([P, 4, HD1], f32, tag="pa", name="pa")
            pt2 = pt.tile([P, 4, P], bf, tag="ptr", name="ptv")
            vns = [None] * nqt

            def mk_mm(qt):
                def go():
                    pa = pa2[:, qt % 4, :]
                    for kb in range(NKB):
                        nc.tensor.matmul(
                            pa,
                            lhsT=et[:, kb, qt * P:(qt + 1) * P],
                            rhs=v_aug[:, b * NKB + kb, h * HD1:(h + 1) * HD1],
                            start=(kb == 0),
                            stop=(kb == NKB - 1),
                        )
                    rc_ = small.tile([P, 1], f32, tag="rc", name="rc")
                    nc.vector.reciprocal(rc_[:], pa2[:, qt % 4, HD:HD1])
                    vn = small.tile([P, HD], bf, tag="vn", name="vn")
                    nc.vector.tensor_scalar_mul(vn[:], pa2[:, qt % 4, 0:HD], rc_[:])
                    vns[qt] = vn
                return go

            def mk_ev(qt):
                def go():
                    ptv = pt2[:, qt % 4, :]
                    nc.tensor.transpose(ptv[hp:hp + HD], vns[qt][:], ident[:])
                    col = qbase + qt * P
                    nc.vector.tensor_copy(
                        out=valsT[hp:hp + HD, col:col + P],
                        in_=pt2[hp:hp + HD, qt % 4, :],
                    )
                return go

            tasks = []
            for qt in range(nqt):
                tasks.append(mk_mm(qt))
                if qt >= 1:
                    tasks.append(mk_ev(qt - 1))
            tasks.append(mk_ev(nqt - 1))
            return tasks

        def t_ccdma(half, j):
            ccin = ccA_in if half == 0 else ccB_in
            hp = half * HD
            return lambda: nc.sync.dma_start(
                out=ccin[j], in_=valsT[hp:hp + HD, j * ROWS:(j + 1) * ROWS]
            )

        def t_a2a(half):
            ccin, ccout = (ccA_in, ccA_out) if half == 0 else (ccB_in, ccB_out)

            def go():
                if local_a2a:
                    # stand-in for the real AllToAll in loop-timing builds;
                    # p-leading layout prices it like the real collective
                    # (~3.2us vs ~3.4us measured for a 0.5MB AllToAll)
                    nc.sync.dma_start(
                        out=ccout.rearrange("i p r -> p i r"),
                        in_=ccin.rearrange("i p r -> p i r"),
                    )
                else:
                    nc.gpsimd.collective_compute(
                        "AllToAll",
                        mybir.AluOpType.bypass,
                        replica_groups=[list(range(NCORES))],
                        ins=[ccin[:]],
                        outs=[ccout[:]],
                    )
            return go

        def t_vfull(half):
            ccout = ccA_out if half == 0 else ccB_out
            hp = half * HD
            return lambda: nc.sync.dma_start(
                out=vfull[hp:hp + HD, :, :],
                in_=ccout.rearrange("i p r -> p i r"),
            )

        def t_oproj(rt, dh):
            def go():
                po = pscore.tile([P, 512], f32, tag="ps", name="po")
                for c in range(NCORES):
                    nc.tensor.matmul(
                        po[:],
                        lhsT=vfull[:, c, rt * P:(rt + 1) * P],
                        rhs=wo_sb[:, c, dh * 512:(dh + 1) * 512],
                        start=(c == 0),
                        stop=(c == NCORES - 1 and not with_bias),
                    )
                if with_bias:
                    nc.tensor.matmul(
                        po[:], lhsT=ones_row[:, 0:P],
                        rhs=ob_sb[:, dh * 512:(dh + 1) * 512],
                        start=False, stop=True,
                    )
                osb = outp.tile([P, 512], f32, tag="osb", name="osb")
                nc.vector.tensor_copy(out=osb[:], in_=po[:])
                nc.sync.dma_start(
                    out=y[rt * P:(rt + 1) * P, dh * 512:(dh + 1) * 512],
                    in_=osb[:],
                )
            return go

        # ---------------- emission (software pipeline) ----------------
        def emit_body(load_weights):
            # front: first x chunk + k/q projections for the first block's
            # initial key blocks and query columns.  DMA queue order matters:
            # the first score matmul waits on x chunk 0 + wk + wq, so those
            # go first; wv and the remaining x chunks follow.
            t_xload(0, 0)()
            if load_weights:
                t_wload(wk_sb, wk)()
                t_wload(wq_sb, wq)()
            t_xload(0, 1)()
            if load_weights:
                t_wload(wv_sb, wv)()
                if with_bias:
                    t_bias_loads()()
            t_kqproj(0, "k", 0)()
            t_kqproj(0, "q", 0)()
            if qcw > 512:
                t_kqproj(0, "q", 1)()
            t_xload(0, 2)()
            t_xload(0, 3)()

            block_ids = [(b, h, qc) for b in range(B) for h in range(HL)
                         for qc in range(nqc)]
            nblk = len(block_ids)

            from collections import defaultdict
            extras = defaultdict(list)       # before the prev block's AVs
            extras_late = defaultdict(list)  # after the prev block's AVs
            tail_tasks = []
            weave = defaultdict(list)        # woven into score tasks: (pos, task)

            def sched(idx, task, late=True):
                if idx < nblk:
                    (extras_late if late else extras)[idx].append(task)
                else:
                    tail_tasks.append(task)

            # cc DMAs: slot j's data comes from block blkX's AV tasks, which
            # run as part of block blkX+1's mix -> schedule the DMA late in
            # block blkX+1 (after those AVs).
            lastA = 0
            for b in range(B):
                for q in range(4):
                    j = b * 4 + q
                    qc_of = q * 512 // qcw
                    blkA = b * nbb + qc_of
                    blkB = b * nbb + nqc + qc_of
                    sched(blkA + 1, t_ccdma(0, j))
                    sched(blkB + 1, t_ccdma(1, j))
                    lastA = max(lastA, blkA + 1)
            sched(lastA, t_a2a(0))
            sched(lastA, t_vfull(0))
            tail_tasks += [t_a2a(1), t_vfull(1)]
            if load_weights:
                sched(nbb, t_wload(wo_sb, wo))

            if nqc == 2:
                # hand schedule for qcw=1024: batch-1 prep deferred into the
                # Act-bound later blocks of batch 0 so the PE-heavy prep
                # doesn't starve the Activation engine early on.
                sched(1, t_xload(1, 0), late=False)
                sched(1, t_xload(1, 1), late=False)
                sched(2, t_xload(1, 2), late=False)
                sched(2, t_xload(1, 3), late=False)
                for t in (t_kqproj(1, "k", 0), t_kqproj(1, "q", 0),
                          t_kqproj(1, "q", 1)):
                    sched(3, t, late=False)
                for st in range(NKB, NKB + 8):
                    sched(3, t_vproj(st))
                # woven into block 4 (b1,h0,qc0): k chunk c before score
                # kb 4c, remaining b1 v tiles spread along
                weave[4] = [
                    (4, t_kqproj(1, "k", 1)), (5, t_vproj(NKB + 8)),
                    (6, t_vproj(NKB + 9)), (7, t_vproj(NKB + 10)),
                    (8, t_kqproj(1, "k", 2)), (9, t_vproj(NKB + 11)),
                    (10, t_vproj(NKB + 12)), (11, t_vproj(NKB + 13)),
                    (12, t_kqproj(1, "k", 3)), (13, t_vproj(NKB + 14)),
                    (14, t_vproj(NKB + 15)),
                ]
                sched(4, t_kqproj(1, "q", 2))
                sched(4, t_kqproj(1, "q", 3))
            else:
                # generic fallback (qcw=512)
                for b in range(B):
                    for c in range(4):
                        first_qc = c * 512 // qcw
                        if b == 0 and first_qc == 0:
                            continue
                        if b == 0:
                            extras[first_qc - 1].insert(0, t_kqproj(b, "q", c))
                b1_tasks = []
                for rc in range(4):
                    b1_tasks.append(t_xload(1, rc))
                for rc in range(4):
                    b1_tasks.append(t_kqproj(1, "k", rc))
                    b1_tasks.append(t_kqproj(1, "q", rc))
                for st in range(NKB, 2 * NKB):
                    b1_tasks.append(t_vproj(st))
                nspread = nbb - 1
                for i, task in enumerate(b1_tasks):
                    sched(1 + i * nspread // len(b1_tasks), task, late=False)

            prev_av = []
            for i, (b, h, qc) in enumerate(block_ids):
                et = expp.tile([P, NKB, qcw], bf, tag="exp", name="et")
                s = score_tasks(b, h, qc, et)
                if i == 0:
                    # weave k chunks 1-3, v tiles 0..15 and (for qcw=1024)
                    # q chunks 2-3 under block 0: score kb 4c..4c+3 needs
                    # k chunk c; AV (next block) needs v tiles of batch 0;
                    # block 1 needs q chunks 2-3.  Spread so the last score
                    # tasks aren't followed by a long prep burst.
                    kp = [t_kqproj(0, "k", c) for c in (1, 2, 3)]
                    vp = [t_vproj(st) for st in range(NKB)]
                    qx = ([t_kqproj(0, "q", 2), t_kqproj(0, "q", 3)]
                          if nqc == 2 else [])
                    primary = (s[0:4] + kp[0:1] + vp[0:3]
                               + s[4:8] + kp[1:2] + vp[3:7]
                               + s[8:10] + qx[0:1] + vp[7:9]
                               + s[10:12] + kp[2:3] + vp[9:11] + qx[1:2]
                               + s[12:14] + vp[11:14]
                               + s[14:16] + vp[14:16])
                elif weave.get(i):
                    primary = []
                    wv_ = sorted(weave[i], key=lambda pt_: pt_[0])
                    wi = 0
                    for si, task in enumerate(s):
                        while wi < len(wv_) and wv_[wi][0] <= si:
                            primary.append(wv_[wi][1])
                            wi += 1
                        primary.append(task)
                    primary += [t for _, t in wv_[wi:]]
                else:
                    primary = s
                mix = extras.get(i, [])[:] + prev_av + extras_late.get(i, [])
                _interleave(primary, mix, lead=2)
                prev_av = av_tasks(b, h, qc, et)
            # final flush: the last block's cc DMAs only need the AV tasks
            # for their own row slots, so interleave them
            if nqc == 2 and len(tail_tasks) == 4:
                # prev_av is [mm0,mm1,ev0,...]; ev3 (slot 6's last row tile)
                # is at index 8
                flush = (prev_av[0:9] + tail_tasks[0:1] + prev_av[9:]
                         + tail_tasks[1:])
            else:
                flush = prev_av + tail_tasks
            for task in flush:
                task()


            # ---- output projection ----
            for rt in range(ROWS // P):
                for dh in range(D // 512):
                    t_oproj(rt, dh)()

        if loop_n > 1:
            t_wload(wv_sb, wv)()
            t_wload(wk_sb, wk)()
            t_wload(wq_sb, wq)()
            t_wload(wo_sb, wo)()
            if with_bias:
                t_bias_loads()()
            with tc.For_i(0, loop_n, 1):
                emit_body(load_weights=False)
        else:
            for rep in range(repeats):
                emit_body(load_weights=(rep == 0))

    nc.compile()
    return nc


def get_program(with_bias: bool, local_a2a: bool = False,
                repeats: int = 1, loop_n: int = 0, qcw: int = QCW, **_ignored):
    key = (with_bias, local_a2a, repeats, loop_n, qcw)
    if key not in _CACHE:
        _CACHE[key] = _build_program(with_bias, local_a2a, repeats, loop_n, qcw)
    return _CACHE[key]


def make_in_maps(x, qkv_w, qkv_b, o_w, o_b):
    """Host-side sharding: slice per-head weight rows, transpose x and the
    weights to the layouts the kernel consumes, cast to bf16."""
    bfnp = ml_dtypes.bfloat16
    x2 = np.asarray(x, np.float32).reshape(BS, D)
    # xT [128, NDC, BS]: element (p, c, r) = x[r, c*128 + p]
    xT = np.ascontiguousarray(
        x2.T.reshape(NDC, P, BS).transpose(1, 0, 2).astype(bfnp)
    )

    qkv_w = np.asarray(qkv_w, np.float32)
    o_w = np.asarray(o_w, np.float32)
    qkv_b = np.asarray(qkv_b, np.float32)
    o_b = np.asarray(o_b, np.float32)

    with_bias = bool(np.any(qkv_b) or np.any(o_b))

    woT = np.ascontiguousarray(
        o_w.T.reshape(NCORES, P, D).transpose(1, 0, 2).astype(bfnp)
    )
    ob_host = np.ascontiguousarray(o_b.reshape(1, D).astype(bfnp))

    in_maps = []
    for m in range(NCORES):
        heads = [m * HL + h for h in range(HL)]
        q_rows = np.concatenate([qkv_w[h * 3 * HD:h * 3 * HD + HD] for h in heads])
        k_rows = np.concatenate(
            [qkv_w[h * 3 * HD + HD:h * 3 * HD + 2 * HD] for h in heads]
        )
        v_rows = np.concatenate(
            [qkv_w[h * 3 * HD + 2 * HD:h * 3 * HD + 3 * HD] for h in heads]
        )

        def wt(rows):
            # [CH, D] -> [D, CH] -> [p, chunk, CH]
            return np.ascontiguousarray(
                rows.T.reshape(NDC, P, CH).transpose(1, 0, 2).astype(bfnp)
            )

        im = {
            "xT": xT,
            "wq": wt(q_rows),
            "wk": wt(k_rows),
            "wv": wt(v_rows),
            "wo": woT,
        }
        if with_bias:
            bqv = np.concatenate(
                [qkv_b[h * 3 * HD:h * 3 * HD + HD] for h in heads]
            )
            bkv = np.concatenate(
                [qkv_b[h * 3 * HD + HD:h * 3 * HD + 2 * HD] for h in heads]
            )
            bvv = np.concatenate(
                [qkv_b[h * 3 * HD + 2 * HD:h * 3 * HD + 3 * HD] for h in heads]
            )
            im["bq"] = np.ascontiguousarray(bqv.reshape(1, CH).astype(bfnp))
            im["bk"] = np.ascontiguousarray(bkv.reshape(1, CH).astype(bfnp))
            im["bv"] = np.ascontiguousarray(bvv.reshape(1, CH).astype(bfnp))
            im["ob"] = ob_host
        in_maps.append(im)
    return in_maps, with_bias


def kernel(x, qkv_w, qkv_b, o_w, o_b):
    from concourse.bass_utils import run_bass_kernel_spmd

    in_maps, with_bias = make_in_maps(x, qkv_w, qkv_b, o_w, o_b)
    nc = get_program(with_bias)
    res = run_bass_kernel_spmd(nc, in_maps, list(range(NCORES)))
    out = np.concatenate([res.results[m]["y"] for m in range(NCORES)], axis=0)
    return np.ascontiguousarray(out.reshape(B, S, D))


# revision 21
# speedup vs baseline: 1.2328x; 1.0463x over previous
"""Multi-head attention (B=2, S=2048, D=1024, H=16) on 8 TRN2 NeuronCores.

Sharding: tensor-parallel over heads (2 heads/core).  Each core computes
the qkv projection for its heads (full sequence) and attention, then an
AllToAll redistributes attention outputs so each core holds *all* heads
for a 1/8 slice of the (batch*seq) rows and runs the output projection
locally.  No cross-core reduction needed.

x is pre-transposed AND pre-cast to bf16 on the host (xT [128, D/128,
B*S]) so the device does no activation transpose at all — projections
read xT directly as the matmul moving operand.  Softmax denominators
come from a ones-column appended to V (scores are small, so exp without
max-subtraction is safe); normalization is fused into the AV eviction.

Compute dtype: bf16 matmul operands, fp32 PSUM accumulation.  Score
chunks are 1024 queries wide so each Exp activation instruction covers
[128, 1024], halving Act-engine instruction overhead vs 512-wide.
"""

import sys

sys.path.insert(0, "/opt/trn_rl_repo")

import numpy as np
import ml_dtypes

B, S, D = 2, 2048, 1024
H, HD = 16, 64
NCORES = 8
BS = B * S                 # 4096 flattened rows
HL = H // NCORES           # 2 local heads
CH = HL * HD               # 128 local q/k/v channels
ROWS = BS // NCORES        # 512 output rows per core
P = 128
NDC = D // P               # 8 chunks of the contraction dim D
NKB = S // P               # 16 key blocks per batch
QCW = 1024                 # query-chunk width (one exp instruction per kb)
HD1 = HD + 1               # value channels + ones column

_CACHE = {}


def _interleave(primary, secondary, lead=0):
    """Emit primary tasks in order, spreading secondary tasks between them.
    The first `lead` primary tasks are emitted before any secondary."""
    ns = len(secondary)
    npr = max(len(primary) - lead, 1)
    si = 0
    for i, p in enumerate(primary):
        p()
        tgt = (i + 1 - lead) * ns // npr if i >= lead else 0
        while si < tgt:
            secondary[si]()
            si += 1
    while si < ns:
        secondary[si]()
        si += 1


def _build_program(with_bias: bool, local_a2a: bool = False,
                   repeats: int = 1, loop_n: int = 0, qcw: int = QCW):
    import concourse.bass as bass
    import concourse.mybir as mybir
    import concourse.tile as tile
    from concourse import bacc
    from concourse.masks import make_identity
    from contextlib import ExitStack

    nqc = S // qcw             # query chunks per batch (2 for qcw=1024)
    nbb = HL * nqc             # blocks per batch (4)
    nqt = qcw // P             # AV row-tiles per block (8)
    dt = mybir.dt
    AF = mybir.ActivationFunctionType
    bf, f32 = dt.bfloat16, dt.float32

    nc = bacc.Bacc()

    xT_d = nc.dram_tensor("xT", [P, NDC, BS], bf, kind="ExternalInput")
    wq = nc.dram_tensor("wq", [P, NDC, CH], bf, kind="ExternalInput")
    wk = nc.dram_tensor("wk", [P, NDC, CH], bf, kind="ExternalInput")
    wv = nc.dram_tensor("wv", [P, NDC, CH], bf, kind="ExternalInput")
    wo = nc.dram_tensor("wo", [P, NCORES, D], bf, kind="ExternalInput")
    if with_bias:
        bq = nc.dram_tensor("bq", [1, CH], bf, kind="ExternalInput")
        bk = nc.dram_tensor("bk", [1, CH], bf, kind="ExternalInput")
        bv = nc.dram_tensor("bv", [1, CH], bf, kind="ExternalInput")
        ob = nc.dram_tensor("ob", [1, D], bf, kind="ExternalInput")
    y = nc.dram_tensor("y", [ROWS, D], f32, kind="ExternalOutput")

    with tile.TileContext(nc) as tc, ExitStack() as ctx:
        const = ctx.enter_context(tc.tile_pool(name="const", bufs=1))
        ident = const.tile([P, P], bf)
        make_identity(nc, ident[:])

        wq_sb = const.tile([P, NDC, CH], bf)
        wk_sb = const.tile([P, NDC, CH], bf)
        wv_sb = const.tile([P, NDC, CH], bf)
        wo_sb = const.tile([P, NCORES, D], bf)
        if with_bias:
            bq_sb = const.tile([1, CH], bf)
            bk_sb = const.tile([1, CH], bf)
            bv_sb = const.tile([1, CH], bf)
            ob_sb = const.tile([1, D], bf)
            ones_row = const.tile([1, 512], bf)

        big = ctx.enter_context(tc.tile_pool(name="big", bufs=1))
        xT = big.tile([P, NDC, BS], bf)                     # [d%128, d//128, row]
        qT = big.tile([P, BS], bf)                          # q channel-major
        kT = big.tile([P, BS], bf)                          # k channel-major
        v_aug = big.tile([P, B * NKB, HL * HD1], bf)        # v row-major + ones
        valsT = big.tile([P, BS], bf)                       # attn out, ch-major
        vfull = big.tile([P, NCORES, ROWS], bf)             # gathered, for oproj
        expp = ctx.enter_context(tc.tile_pool(name="expp", bufs=2))
        small = ctx.enter_context(tc.tile_pool(name="small", bufs=4))
        outp = ctx.enter_context(tc.tile_pool(name="outp", bufs=2))

        # PSUM budget (8 banks): pscore 2 bufs x 2 banks (1024-wide f32),
        # pbig 2 x 1 (projections), pav 1, pt 1 (AV transpose staging)
        pt = ctx.enter_context(tc.tile_pool(name="pt", bufs=1, space="PSUM"))
        pbig = ctx.enter_context(tc.tile_pool(name="pbig", bufs=2, space="PSUM"))
        pscore = ctx.enter_context(tc.tile_pool(name="pscore", bufs=2, space="PSUM"))
        pav = ctx.enter_context(tc.tile_pool(name="pav", bufs=1, space="PSUM"))

        dram = ctx.enter_context(tc.tile_pool(name="dram", bufs=1, space="DRAM"))
        # the AllToAll is split into two half-payload collectives (head 0 /
        # head 1 channel halves) so the first can run under live attention
        ccA_in = dram.tile([NCORES, HD, ROWS], bf)
        ccA_out = dram.tile([NCORES, HD, ROWS], bf)
        ccB_in = dram.tile([NCORES, HD, ROWS], bf)
        ccB_out = dram.tile([NCORES, HD, ROWS], bf)

        # ones columns for the softmax-denominator trick; value columns are
        # overwritten by the v-projection evictions
        for h in range(HL):
            nc.vector.memset(v_aug[:, :, h * HD1 + HD], 1.0)

        # ---------------- task builders ----------------

        def t_wload(wsb, wdram):
            return lambda: nc.sync.dma_start(out=wsb[:], in_=wdram[:])

        def t_bias_loads():
            def go():
                nc.sync.dma_start(out=bq_sb[:], in_=bq[:])
                nc.sync.dma_start(out=bk_sb[:], in_=bk[:])
                nc.sync.dma_start(out=bv_sb[:], in_=bv[:])
                nc.sync.dma_start(out=ob_sb[:], in_=ob[:])
                nc.vector.memset(ones_row[:], 1.0)
            return go

        def t_xload(b, rc):
            # one 512-column chunk of xT, all depth slices
            def go():
                c0 = b * S + rc * 512
                nc.sync.dma_start(
                    out=xT[:, :, c0:c0 + 512], in_=xT_d[:, :, c0:c0 + 512]
                )
            return go

        def t_vproj(st):
            def go():
                pv = pbig.tile([P, CH], f32, tag="pk", name="pv")
                for c in range(NDC):
                    nc.tensor.matmul(
                        pv[:],
                        lhsT=xT[:, c, st * P:(st + 1) * P],
                        rhs=wv_sb[:, c, :],
                        start=(c == 0),
                        stop=(c == NDC - 1 and not with_bias),
                    )
                if with_bias:
                    nc.tensor.matmul(
                        pv[:], lhsT=ones_row[:, 0:P], rhs=bv_sb[:],
                        start=False, stop=True,
                    )
                for h in range(HL):
                    nc.vector.tensor_copy(
                        out=v_aug[:, st, h * HD1:h * HD1 + HD],
                        in_=pv[:, h * HD:(h + 1) * HD],
                    )
            return go

        def t_kqproj(b, which, qc):
            # qc indexes 512-wide column chunks (0..3 per batch)
            def go():
                wsb, dst = (wk_sb, kT) if which == "k" else (wq_sb, qT)
                base = b * S + qc * 512
                pq = pbig.tile([P, 512], f32, tag="pk", name="pq")
                for c in range(NDC):
                    nc.tensor.matmul(
                        pq[:],
                        lhsT=wsb[:, c, :],
                        rhs=xT[:, c, base:base + 512],
                        start=(c == 0),
                        stop=(c == NDC - 1 and not with_bias),
                    )
                if with_bias:
                    nc.tensor.matmul(
                        pq[:],
                        lhsT=(bk_sb if which == "k" else bq_sb)[:],
                        rhs=ones_row[:],
                        start=False, stop=True,
                    )
                nc.vector.tensor_copy(out=dst[:, base:base + 512], in_=pq[:])
            return go

        # attention blocks: per (b, h, qc) -> score tasks (one per kb) and
        # AV tasks (one per 128-query tile)
        def score_tasks(b, h, qc, et):
            hp = h * HD
            qbase = b * S + qc * qcw
            tasks = []

            def mk(kb):
                def go():
                    kbase = b * S + kb * P
                    ps = pscore.tile([P, qcw], f32, tag="ps", name="ps")
                    for qh in range(qcw // 512):
                        nc.tensor.matmul(
                            ps[:, qh * 512:(qh + 1) * 512],
                            lhsT=kT[hp:hp + HD, kbase:kbase + P],
                            rhs=qT[hp:hp + HD,
                                   qbase + qh * 512:qbase + (qh + 1) * 512],
                            start=True,
                            stop=True,
                        )
                    nc.scalar.activation(et[:, kb, :], ps[:], AF.Exp, scale=0.125)
                return go

            for kb in range(NKB):
                tasks.append(mk(kb))
            return tasks

        def av_tasks(b, h, qc, et):
            """Returns interleaved [mm0, mm1, ev0, mm2, ev1, ...]: the PE
            transpose in ev_k waits on a DVE chain, so it is emitted one
            AV-tile later than its matmuls to hide the cross-engine
            latency.  4 sub-slices inside the single pav/pt banks keep the
            tiles independent."""
            hp = h * HD
            qbase = b * S + qc * qcw
            pa2 = pav.tile([P, 4, HD1], f32, tag="pa", name="pa")
            pt2 = pt.tile([P, 4, P], bf, tag="ptr", name="ptv")
            vns = [None] * nqt

            def mk_mm(qt):
                def go():
                    pa = pa2[:, qt % 4, :]
                    for kb in range(NKB):
                        nc.tensor.matmul(
                            pa,
                            lhsT=et[:, kb, qt * P:(qt + 1) * P],
                            rhs=v_aug[:, b * NKB + kb, h * HD1:(h + 1) * HD1],
                            start=(kb == 0),
                            stop=(kb == NKB - 1),
                        )
                    rc_ = small.tile([P, 1], f32, tag="rc", name="rc")
                    nc.vector.reciprocal(rc_[:], pa2[:, qt % 4, HD:HD1])
                    vn = small.tile([P, HD], bf, tag="vn", name="vn")
                    nc.vector.tensor_scalar_mul(vn[:], pa2[:, qt % 4, 0:HD], rc_[:])
                    vns[qt] = vn
                return go

            def mk_ev(qt):
                def go():
                    ptv = pt2[:, qt % 4, :]
                    nc.tensor.transpose(ptv[hp:hp + HD], vns[qt][:], ident[:])
                    col = qbase + qt * P
                    nc.vector.tensor_copy(
                        out=valsT[hp:hp + HD, col:col + P],
                        in_=pt2[hp:hp + HD, qt % 4, :],
                    )
                return go

            tasks = []
            for qt in range(nqt):
                tasks.append(mk_mm(qt))
                if qt >= 1:
                    tasks.append(mk_ev(qt - 1))
            tasks.append(mk_ev(nqt - 1))
            return tasks

        def t_ccdma(half, j):
            ccin = ccA_in if half == 0 else ccB_in
            hp = half * HD
            return lambda: nc.sync.dma_start(
                out=ccin[j], in_=valsT[hp:hp + HD, j * ROWS:(j + 1) * ROWS]
            )

        def t_a2a(half):
            ccin, ccout = (ccA_in, ccA_out) if half == 0 else (ccB_in, ccB_out)

            def go():
                if local_a2a:
                    # stand-in for the real AllToAll in loop-timing builds;
                    # p-leading layout prices it like the real collective
                    # (~3.2us vs ~3.4us measured for a 0.5MB AllToAll)
                    nc.sync.dma_start(
                        out=ccout.rearrange("i p r -> p i r"),
                        in_=ccin.rearrange("i p r -> p i r"),
                    )
                else:
                    nc.gpsimd.collective_compute(
                        "AllToAll",
                        mybir.AluOpType.bypass,
                        replica_groups=[list(range(NCORES))],
                        ins=[ccin[:]],
                        outs=[ccout[:]],
                    )
            return go

        def t_vfull(half):
            ccout = ccA_out if half == 0 else ccB_out
            hp = half * HD
            return lambda: nc.sync.dma_start(
                out=vfull[hp:hp + HD, :, :],
                in_=ccout.rearrange("i p r -> p i r"),
            )

        def t_oproj(rt, dh):
            def go():
                po = pscore.tile([P, 512], f32, tag="ps", name="po")
                for c in range(NCORES):
                    nc.tensor.matmul(
                        po[:],
                        lhsT=vfull[:, c, rt * P:(rt + 1) * P],
                        rhs=wo_sb[:, c, dh * 512:(dh + 1) * 512],
                        start=(c == 0),
                        stop=(c == NCORES - 1 and not with_bias),
                    )
                if with_bias:
                    nc.tensor.matmul(
                        po[:], lhsT=ones_row[:, 0:P],
                        rhs=ob_sb[:, dh * 512:(dh + 1) * 512],
                        start=False, stop=True,
                    )
                osb = outp.tile([P, 512], f32, tag="osb", name="osb")
                nc.vector.tensor_copy(out=osb[:], in_=po[:])
                nc.sync.dma_start(
                    out=y[rt * P:(rt + 1) * P, dh * 512:(dh + 1) * 512],
                    in_=osb[:],
                )
            return go

        # ---------------- emission (software pipeline) ----------------
        def emit_body(load_weights, pipelined_x=False):
            # front: first x chunk + k/q projections for the first block's
            # initial key blocks and query columns.  DMA queue order matters:
            # the first score matmul waits on x chunk 0 + wk + wq, so those
            # go first; wv and the remaining x chunks follow.  In pipelined
            # mode (hardware loop) the x chunks were loaded near the end of
            # the previous iteration, so the projections start immediately.
            if not pipelined_x:
                t_xload(0, 0)()
            if load_weights:
                t_wload(wk_sb, wk)()
                t_wload(wq_sb, wq)()
            if not pipelined_x:
                t_xload(0, 1)()
            if load_weights:
                t_wload(wv_sb, wv)()
                if with_bias:
                    t_bias_loads()()
            t_kqproj(0, "k", 0)()
            t_kqproj(0, "q", 0)()
            if qcw > 512:
                t_kqproj(0, "q", 1)()
            if not pipelined_x:
                t_xload(0, 2)()
                t_xload(0, 3)()

            block_ids = [(b, h, qc) for b in range(B) for h in range(HL)
                         for qc in range(nqc)]
            nblk = len(block_ids)

            from collections import defaultdict
            extras = defaultdict(list)       # before the prev block's AVs
            extras_late = defaultdict(list)  # after the prev block's AVs
            tail_tasks = []
            weave = defaultdict(list)        # woven into score tasks: (pos, task)

            def sched(idx, task, late=True):
                if idx < nblk:
                    (extras_late if late else extras)[idx].append(task)
                else:
                    tail_tasks.append(task)

            # cc DMAs: slot j's data comes from block blkX's AV tasks, which
            # run as part of block blkX+1's mix -> schedule the DMA late in
            # block blkX+1 (after those AVs).
            lastA = 0
            for b in range(B):
                for q in range(4):
                    j = b * 4 + q
                    qc_of = q * 512 // qcw
                    blkA = b * nbb + qc_of
                    blkB = b * nbb + nqc + qc_of
                    sched(blkA + 1, t_ccdma(0, j))
                    sched(blkB + 1, t_ccdma(1, j))
                    lastA = max(lastA, blkA + 1)
            sched(lastA, t_a2a(0))
            sched(lastA, t_vfull(0))
            tail_tasks += [t_a2a(1), t_vfull(1)]
            if load_weights:
                sched(nbb, t_wload(wo_sb, wo))

            if nqc == 2:
                # hand schedule for qcw=1024: batch-1 prep deferred into the
                # Act-bound later blocks of batch 0 so the PE-heavy prep
                # doesn't starve the Activation engine early on.
                if pipelined_x:
                    # reload both batches' x chunks for the NEXT iteration
                    # once all of this iteration's xT readers are done
                    # (the last are batch-1 k/q/v projections in block 4)
                    for rc in range(4):
                        sched(5, t_xload(0, rc))
                        sched(6, t_xload(1, rc))
                else:
                    sched(1, t_xload(1, 0), late=False)
                    sched(1, t_xload(1, 1), late=False)
                    sched(2, t_xload(1, 2), late=False)
                    sched(2, t_xload(1, 3), late=False)
                for t in (t_kqproj(1, "k", 0), t_kqproj(1, "q", 0),
                          t_kqproj(1, "q", 1)):
                    sched(3, t, late=False)
                for st in range(NKB, NKB + 8):
                    sched(3, t_vproj(st))
                # woven into block 4 (b1,h0,qc0): k chunk c before score
                # kb 4c, remaining b1 v tiles spread along
                weave[4] = [
                    (4, t_kqproj(1, "k", 1)), (5, t_vproj(NKB + 8)),
                    (6, t_vproj(NKB + 9)), (7, t_vproj(NKB + 10)),
                    (8, t_kqproj(1, "k", 2)), (9, t_vproj(NKB + 11)),
                    (10, t_vproj(NKB + 12)), (11, t_vproj(NKB + 13)),
                    (12, t_kqproj(1, "k", 3)), (13, t_vproj(NKB + 14)),
                    (14, t_vproj(NKB + 15)),
                ]
                sched(4, t_kqproj(1, "q", 2))
                sched(4, t_kqproj(1, "q", 3))
            else:
                # generic fallback (qcw=512)
                for b in range(B):
                    for c in range(4):
                        first_qc = c * 512 // qcw
                        if b == 0 and first_qc == 0:
                            continue
                        if b == 0:
                            extras[first_qc - 1].insert(0, t_kqproj(b, "q", c))
                b1_tasks = []
                for rc in range(4):
                    b1_tasks.append(t_xload(1, rc))
                for rc in range(4):
                    b1_tasks.append(t_kqproj(1, "k", rc))
                    b1_tasks.append(t_kqproj(1, "q", rc))
                for st in range(NKB, 2 * NKB):
                    b1_tasks.append(t_vproj(st))
                nspread = nbb - 1
                for i, task in enumerate(b1_tasks):
                    sched(1 + i * nspread // len(b1_tasks), task, late=False)

            prev_av = []
            for i, (b, h, qc) in enumerate(block_ids):
                et = expp.tile([P, NKB, qcw], bf, tag="exp", name="et")
                s = score_tasks(b, h, qc, et)
                if i == 0:
                    # weave k chunks 1-3, v tiles 0..15 and (for qcw=1024)
                    # q chunks 2-3 under block 0: score kb 4c..4c+3 needs
                    # k chunk c; AV (next block) needs v tiles of batch 0;
                    # block 1 needs q chunks 2-3.  Spread so the last score
                    # tasks aren't followed by a long prep burst.
                    kp = [t_kqproj(0, "k", c) for c in (1, 2, 3)]
                    vp = [t_vproj(st) for st in range(NKB)]
                    qx = ([t_kqproj(0, "q", 2), t_kqproj(0, "q", 3)]
                          if nqc == 2 else [])
                    primary = (s[0:4] + kp[0:1] + vp[0:3]
                               + s[4:8] + kp[1:2] + vp[3:7]
                               + s[8:10] + qx[0:1] + vp[7:9]
                               + s[10:12] + kp[2:3] + vp[9:11] + qx[1:2]
                               + s[12:14] + vp[11:14]
                               + s[14:16] + vp[14:16])
                elif weave.get(i):
                    primary = []
                    wv_ = sorted(weave[i], key=lambda pt_: pt_[0])
                    wi = 0
                    for si, task in enumerate(s):
                        while wi < len(wv_) and wv_[wi][0] <= si:
                            primary.append(wv_[wi][1])
                            wi += 1
                        primary.append(task)
                    primary += [t for _, t in wv_[wi:]]
                else:
                    primary = s
                mix = extras.get(i, [])[:] + prev_av + extras_late.get(i, [])
                _interleave(primary, mix, lead=2)
                prev_av = av_tasks(b, h, qc, et)
            # final flush: the last block's cc DMAs only need the AV tasks
            # for their own row slots, so interleave them
            if nqc == 2 and len(tail_tasks) == 4:
                # prev_av is [mm0,mm1,ev0,...]; ev3 (slot 6's last row tile)
                # is at index 8
                flush = (prev_av[0:9] + tail_tasks[0:1] + prev_av[9:]
                         + tail_tasks[1:])
            else:
                flush = prev_av + tail_tasks
            for task in flush:
                task()


            # ---- output projection ----
            for rt in range(ROWS // P):
                for dh in range(D // 512):
                    t_oproj(rt, dh)()

        if loop_n > 1:
            t_wload(wk_sb, wk)()
            t_wload(wq_sb, wq)()
            t_wload(wv_sb, wv)()
            t_wload(wo_sb, wo)()
            if with_bias:
                t_bias_loads()()
            pipelined = (nqc == 2)
            if pipelined:
                for b in range(B):
                    for rc in range(4):
                        t_xload(b, rc)()
            with tc.For_i(0, loop_n, 1):
                emit_body(load_weights=False, pipelined_x=pipelined)
        else:
            for rep in range(repeats):
                emit_body(load_weights=(rep == 0))

    nc.compile()
    return nc


def get_program(with_bias: bool, local_a2a: bool = False,
                repeats: int = 1, loop_n: int = 0, qcw: int = QCW, **_ignored):
    key = (with_bias, local_a2a, repeats, loop_n, qcw)
    if key not in _CACHE:
        _CACHE[key] = _build_program(with_bias, local_a2a, repeats, loop_n, qcw)
    return _CACHE[key]


def make_in_maps(x, qkv_w, qkv_b, o_w, o_b):
    """Host-side sharding: slice per-head weight rows, transpose x and the
    weights to the layouts the kernel consumes, cast to bf16."""
    bfnp = ml_dtypes.bfloat16
    x2 = np.asarray(x, np.float32).reshape(BS, D)
    # xT [128, NDC, BS]: element (p, c, r) = x[r, c*128 + p]
    xT = np.ascontiguousarray(
        x2.T.reshape(NDC, P, BS).transpose(1, 0, 2).astype(bfnp)
    )

    qkv_w = np.asarray(qkv_w, np.float32)
    o_w = np.asarray(o_w, np.float32)
    qkv_b = np.asarray(qkv_b, np.float32)
    o_b = np.asarray(o_b, np.float32)

    with_bias = bool(np.any(qkv_b) or np.any(o_b))

    woT = np.ascontiguousarray(
        o_w.T.reshape(NCORES, P, D).transpose(1, 0, 2).astype(bfnp)
    )
    ob_host = np.ascontiguousarray(o_b.reshape(1, D).astype(bfnp))

    in_maps = []
    for m in range(NCORES):
        heads = [m * HL + h for h in range(HL)]
        q_rows = np.concatenate([qkv_w[h * 3 * HD:h * 3 * HD + HD] for h in heads])
        k_rows = np.concatenate(
            [qkv_w[h * 3 * HD + HD:h * 3 * HD + 2 * HD] for h in heads]
        )
        v_rows = np.concatenate(
            [qkv_w[h * 3 * HD + 2 * HD:h * 3 * HD + 3 * HD] for h in heads]
        )

        def wt(rows):
            # [CH, D] -> [D, CH] -> [p, chunk, CH]
            return np.ascontiguousarray(
                rows.T.reshape(NDC, P, CH).transpose(1, 0, 2).astype(bfnp)
            )

        im = {
            "xT": xT,
            "wq": wt(q_rows),
            "wk": wt(k_rows),
            "wv": wt(v_rows),
            "wo": woT,
        }
        if with_bias:
            bqv = np.concatenate(
                [qkv_b[h * 3 * HD:h * 3 * HD + HD] for h in heads]
            )
            bkv = np.concatenate(
                [qkv_b[h * 3 * HD + HD:h * 3 * HD + 2 * HD] for h in heads]
            )
            bvv = np.concatenate(
                [qkv_b[h * 3 * HD + 2 * HD:h * 3 * HD + 3 * HD] for h in heads]
            )
            im["bq"] = np.ascontiguousarray(bqv.reshape(1, CH).astype(bfnp))
            im["bk"] = np.ascontiguousarray(bkv.reshape(1, CH).astype(bfnp))
            im["bv"] = np.ascontiguousarray(bvv.reshape(1, CH).astype(bfnp))
            im["ob"] = ob_host
        in_maps.append(im)
    return in_maps, with_bias


def kernel(x, qkv_w, qkv_b, o_w, o_b):
    from concourse.bass_utils import run_bass_kernel_spmd

    in_maps, with_bias = make_in_maps(x, qkv_w, qkv_b, o_w, o_b)
    nc = get_program(with_bias)
    res = run_bass_kernel_spmd(nc, in_maps, list(range(NCORES)))
    out = np.concatenate([res.results[m]["y"] for m in range(NCORES)], axis=0)
    return np.ascontiguousarray(out.reshape(B, S, D))


# revision 24
# speedup vs baseline: 1.3367x; 1.0843x over previous
"""Multi-head attention (B=2, S=2048, D=1024, H=16) on 8 TRN2 NeuronCores.

Sharding: tensor-parallel over heads (2 heads/core).  Each core computes
the qkv projection for its heads (full sequence) and attention, then an
AllToAll redistributes attention outputs so each core holds *all* heads
for a 1/8 slice of the (batch*seq) rows and runs the output projection
locally.  No cross-core reduction needed.

x is pre-transposed AND pre-cast to bf16 on the host (xT [128, D/128,
B*S]) so the device does no activation transpose at all — projections
read xT directly as the matmul moving operand.  Softmax denominators
come from a ones-column appended to V (scores are small, so exp without
max-subtraction is safe); normalization is fused into the AV eviction.

Compute dtype: bf16 matmul operands, fp32 PSUM accumulation.  Score
chunks are 1024 queries wide so each Exp activation instruction covers
[128, 1024], halving Act-engine instruction overhead vs 512-wide.
"""

import sys

sys.path.insert(0, "/opt/trn_rl_repo")

import numpy as np
import ml_dtypes

B, S, D = 2, 2048, 1024
H, HD = 16, 64
NCORES = 8
BS = B * S                 # 4096 flattened rows
HL = H // NCORES           # 2 local heads
CH = HL * HD               # 128 local q/k/v channels
ROWS = BS // NCORES        # 512 output rows per core
P = 128
NDC = D // P               # 8 chunks of the contraction dim D
NKB = S // P               # 16 key blocks per batch
QCW = 1024                 # query-chunk width (one exp instruction per kb)
HD1 = HD + 1               # value channels + ones column

_CACHE = {}


def _interleave(primary, secondary, lead=0):
    """Emit primary tasks in order, spreading secondary tasks between them.
    The first `lead` primary tasks are emitted before any secondary."""
    ns = len(secondary)
    npr = max(len(primary) - lead, 1)
    si = 0
    for i, p in enumerate(primary):
        p()
        tgt = (i + 1 - lead) * ns // npr if i >= lead else 0
        while si < tgt:
            secondary[si]()
            si += 1
    while si < ns:
        secondary[si]()
        si += 1


def _build_program(with_bias: bool, local_a2a: bool = False,
                   repeats: int = 1, loop_n: int = 0, qcw: int = QCW):
    import concourse.bass as bass
    import concourse.mybir as mybir
    import concourse.tile as tile
    from concourse import bacc
    from concourse.masks import make_identity
    from contextlib import ExitStack

    nqc = S // qcw             # query chunks per batch (2 for qcw=1024)
    nbb = HL * nqc             # blocks per batch (4)
    nqt = qcw // P             # AV row-tiles per block (8)
    dt = mybir.dt
    AF = mybir.ActivationFunctionType
    bf, f32 = dt.bfloat16, dt.float32

    nc = bacc.Bacc()

    xT_d = nc.dram_tensor("xT", [P, NDC, BS], bf, kind="ExternalInput")
    wq = nc.dram_tensor("wq", [P, NDC, CH], bf, kind="ExternalInput")
    wk = nc.dram_tensor("wk", [P, NDC, CH], bf, kind="ExternalInput")
    wv = nc.dram_tensor("wv", [P, NDC, CH], bf, kind="ExternalInput")
    wo = nc.dram_tensor("wo", [P, NCORES, D], bf, kind="ExternalInput")
    if with_bias:
        bq = nc.dram_tensor("bq", [1, CH], bf, kind="ExternalInput")
        bk = nc.dram_tensor("bk", [1, CH], bf, kind="ExternalInput")
        bv = nc.dram_tensor("bv", [1, CH], bf, kind="ExternalInput")
        ob = nc.dram_tensor("ob", [1, D], bf, kind="ExternalInput")
    y = nc.dram_tensor("y", [ROWS, D], f32, kind="ExternalOutput")

    with tile.TileContext(nc) as tc, ExitStack() as ctx:
        const = ctx.enter_context(tc.tile_pool(name="const", bufs=1))
        ident = const.tile([P, P], bf)
        make_identity(nc, ident[:])

        wq_sb = const.tile([P, NDC, CH], bf)
        wk_sb = const.tile([P, NDC, CH], bf)
        wv_sb = const.tile([P, NDC, CH], bf)
        wo_sb = const.tile([P, NCORES, D], bf)
        if with_bias:
            bq_sb = const.tile([1, CH], bf)
            bk_sb = const.tile([1, CH], bf)
            bv_sb = const.tile([1, CH], bf)
            ob_sb = const.tile([1, D], bf)
            ones_row = const.tile([1, 512], bf)

        big = ctx.enter_context(tc.tile_pool(name="big", bufs=1))
        xT = big.tile([P, NDC, BS], bf)                     # [d%128, d//128, row]
        qT = big.tile([P, BS], bf)                          # q channel-major
        kT = big.tile([P, BS], bf)                          # k channel-major
        v_aug = big.tile([P, B * NKB, HL * HD1], bf)        # v row-major + ones
        valsT = big.tile([P, BS], bf)                       # attn out, ch-major
        vfull = big.tile([P, NCORES, ROWS], bf)             # gathered, for oproj
        expp = ctx.enter_context(tc.tile_pool(name="expp", bufs=2))
        small = ctx.enter_context(tc.tile_pool(name="small", bufs=4))
        outp = ctx.enter_context(tc.tile_pool(name="outp", bufs=2))

        # PSUM budget (8 banks): pscore 2 bufs x 2 banks (1024-wide f32),
        # pbig 2 x 1 (projections), pav 1, pt 1 (AV transpose staging)
        pt = ctx.enter_context(tc.tile_pool(name="pt", bufs=1, space="PSUM"))
        pbig = ctx.enter_context(tc.tile_pool(name="pbig", bufs=2, space="PSUM"))
        pscore = ctx.enter_context(tc.tile_pool(name="pscore", bufs=2, space="PSUM"))
        pav = ctx.enter_context(tc.tile_pool(name="pav", bufs=1, space="PSUM"))

        dram = ctx.enter_context(tc.tile_pool(name="dram", bufs=1, space="DRAM"))
        # the AllToAll is split into two half-payload collectives (head 0 /
        # head 1 channel halves) so the first can run under live attention
        ccA_in = dram.tile([NCORES, HD, ROWS], bf)
        ccA_out = dram.tile([NCORES, HD, ROWS], bf)
        ccB_in = dram.tile([NCORES, HD, ROWS], bf)
        ccB_out = dram.tile([NCORES, HD, ROWS], bf)

        # ones columns for the softmax-denominator trick; value columns are
        # overwritten by the v-projection evictions
        for h in range(HL):
            nc.vector.memset(v_aug[:, :, h * HD1 + HD], 1.0)

        # ---------------- task builders ----------------

        def t_wload(wsb, wdram):
            return lambda: nc.sync.dma_start(out=wsb[:], in_=wdram[:])

        def t_bias_loads():
            def go():
                nc.sync.dma_start(out=bq_sb[:], in_=bq[:])
                nc.sync.dma_start(out=bk_sb[:], in_=bk[:])
                nc.sync.dma_start(out=bv_sb[:], in_=bv[:])
                nc.sync.dma_start(out=ob_sb[:], in_=ob[:])
                nc.vector.memset(ones_row[:], 1.0)
            return go

        def t_xload(b, rc):
            # one 512-column chunk of xT, all depth slices
            def go():
                c0 = b * S + rc * 512
                nc.sync.dma_start(
                    out=xT[:, :, c0:c0 + 512], in_=xT_d[:, :, c0:c0 + 512]
                )
            return go

        def t_vproj(st):
            def go():
                pv = pbig.tile([P, CH], f32, tag="pk", name="pv")
                for c in range(NDC):
                    nc.tensor.matmul(
                        pv[:],
                        lhsT=xT[:, c, st * P:(st + 1) * P],
                        rhs=wv_sb[:, c, :],
                        start=(c == 0),
                        stop=(c == NDC - 1 and not with_bias),
                    )
                if with_bias:
                    nc.tensor.matmul(
                        pv[:], lhsT=ones_row[:, 0:P], rhs=bv_sb[:],
                        start=False, stop=True,
                    )
                for h in range(HL):
                    nc.vector.tensor_copy(
                        out=v_aug[:, st, h * HD1:h * HD1 + HD],
                        in_=pv[:, h * HD:(h + 1) * HD],
                    )
            return go

        def t_kqproj(b, which, qc):
            # qc indexes 512-wide column chunks (0..3 per batch)
            def go():
                wsb, dst = (wk_sb, kT) if which == "k" else (wq_sb, qT)
                base = b * S + qc * 512
                pq = pbig.tile([P, 512], f32, tag="pk", name="pq")
                for c in range(NDC):
                    nc.tensor.matmul(
                        pq[:],
                        lhsT=wsb[:, c, :],
                        rhs=xT[:, c, base:base + 512],
                        start=(c == 0),
                        stop=(c == NDC - 1 and not with_bias),
                    )
                if with_bias:
                    nc.tensor.matmul(
                        pq[:],
                        lhsT=(bk_sb if which == "k" else bq_sb)[:],
                        rhs=ones_row[:],
                        start=False, stop=True,
                    )
                nc.vector.tensor_copy(out=dst[:, base:base + 512], in_=pq[:])
            return go

        # attention blocks: per (b, h, qc) -> score tasks (one per kb) and
        # AV tasks (one per 128-query tile)
        def score_tasks(b, h, qc, et):
            hp = h * HD
            qbase = b * S + qc * qcw
            tasks = []

            def mk(kb):
                def go():
                    kbase = b * S + kb * P
                    ps = pscore.tile([P, qcw], f32, tag="ps", name="ps")
                    for qh in range(qcw // 512):
                        nc.tensor.matmul(
                            ps[:, qh * 512:(qh + 1) * 512],
                            lhsT=kT[hp:hp + HD, kbase:kbase + P],
                            rhs=qT[hp:hp + HD,
                                   qbase + qh * 512:qbase + (qh + 1) * 512],
                            start=True,
                            stop=True,
                        )
                    nc.scalar.activation(et[:, kb, :], ps[:], AF.Exp, scale=0.125)
                return go

            for kb in range(NKB):
                tasks.append(mk(kb))
            return tasks

        def av_tasks(b, h, qc, et):
            """Returns interleaved [mm0, mm1, ev0, mm2, ev1, ...]: the PE
            transpose in ev_k waits on a DVE chain, so it is emitted one
            AV-tile later than its matmuls to hide the cross-engine
            latency.  4 sub-slices inside the single pav/pt banks keep the
            tiles independent."""
            hp = h * HD
            qbase = b * S + qc * qcw
            pa2 = pav.tile([P, 4, HD1], f32, tag="pa", name="pa")
            pt2 = pt.tile([P, 4, P], bf, tag="ptr", name="ptv")
            vns = [None] * nqt

            def mk_mm(qt):
                def go():
                    pa = pa2[:, qt % 4, :]
                    for kb in range(NKB):
                        nc.tensor.matmul(
                            pa,
                            lhsT=et[:, kb, qt * P:(qt + 1) * P],
                            rhs=v_aug[:, b * NKB + kb, h * HD1:(h + 1) * HD1],
                            start=(kb == 0),
                            stop=(kb == NKB - 1),
                        )
                    rc_ = small.tile([P, 1], f32, tag="rc", name="rc")
                    nc.vector.reciprocal(rc_[:], pa2[:, qt % 4, HD:HD1])
                    vn = small.tile([P, HD], bf, tag="vn", name="vn")
                    nc.vector.tensor_scalar_mul(vn[:], pa2[:, qt % 4, 0:HD], rc_[:])
                    vns[qt] = vn
                return go

            def mk_ev(qt):
                def go():
                    ptv = pt2[:, qt % 4, :]
                    nc.tensor.transpose(ptv[hp:hp + HD], vns[qt][:], ident[:])
                    col = qbase + qt * P
                    nc.vector.tensor_copy(
                        out=valsT[hp:hp + HD, col:col + P],
                        in_=pt2[hp:hp + HD, qt % 4, :],
                    )
                return go

            tasks = []
            for qt in range(nqt):
                tasks.append(mk_mm(qt))
                if qt >= 1:
                    tasks.append(mk_ev(qt - 1))
            tasks.append(mk_ev(nqt - 1))
            return tasks

        def t_ccdma(half, j):
            ccin = ccA_in if half == 0 else ccB_in
            hp = half * HD
            return lambda: nc.sync.dma_start(
                out=ccin[j], in_=valsT[hp:hp + HD, j * ROWS:(j + 1) * ROWS]
            )

        def t_a2a(half):
            ccin, ccout = (ccA_in, ccA_out) if half == 0 else (ccB_in, ccB_out)

            def go():
                if local_a2a:
                    # stand-in for the real AllToAll in loop-timing builds;
                    # p-leading layout prices it like the real collective
                    # (~3.2us vs ~3.4us measured for a 0.5MB AllToAll)
                    nc.sync.dma_start(
                        out=ccout.rearrange("i p r -> p i r"),
                        in_=ccin.rearrange("i p r -> p i r"),
                    )
                else:
                    nc.gpsimd.collective_compute(
                        "AllToAll",
                        mybir.AluOpType.bypass,
                        replica_groups=[list(range(NCORES))],
                        ins=[ccin[:]],
                        outs=[ccout[:]],
                    )
            return go

        def t_vfull(half):
            ccout = ccA_out if half == 0 else ccB_out
            hp = half * HD
            return lambda: nc.sync.dma_start(
                out=vfull[hp:hp + HD, :, :],
                in_=ccout.rearrange("i p r -> p i r"),
            )

        def t_oproj(rt, dh):
            def go():
                po = pscore.tile([P, 512], f32, tag="ps", name="po")
                for c in range(NCORES):
                    nc.tensor.matmul(
                        po[:],
                        lhsT=vfull[:, c, rt * P:(rt + 1) * P],
                        rhs=wo_sb[:, c, dh * 512:(dh + 1) * 512],
                        start=(c == 0),
                        stop=(c == NCORES - 1 and not with_bias),
                    )
                if with_bias:
                    nc.tensor.matmul(
                        po[:], lhsT=ones_row[:, 0:P],
                        rhs=ob_sb[:, dh * 512:(dh + 1) * 512],
                        start=False, stop=True,
                    )
                osb = outp.tile([P, 512], f32, tag="osb", name="osb")
                nc.vector.tensor_copy(out=osb[:], in_=po[:])
                nc.sync.dma_start(
                    out=y[rt * P:(rt + 1) * P, dh * 512:(dh + 1) * 512],
                    in_=osb[:],
                )
            return go

        # ---------------- emission (software pipeline) ----------------
        def emit_body(load_weights, pipelined_x=False):
            # front: first x chunk + k/q projections for the first block's
            # initial key blocks and query columns.  DMA queue order matters:
            # the first score matmul waits on x chunk 0 + wk + wq, so those
            # go first; wv and the remaining x chunks follow.  In pipelined
            # mode (hardware loop) the x chunks were loaded near the end of
            # the previous iteration, so the projections start immediately.
            if not pipelined_x:
                t_xload(0, 0)()
            if load_weights:
                t_wload(wk_sb, wk)()
                t_wload(wq_sb, wq)()
            if not pipelined_x:
                t_xload(0, 1)()
            if load_weights:
                t_wload(wv_sb, wv)()
                if with_bias:
                    t_bias_loads()()
            if not pipelined_x:
                t_kqproj(0, "k", 0)()
                t_kqproj(0, "q", 0)()
                if qcw > 512:
                    t_kqproj(0, "q", 1)()
                t_xload(0, 2)()
                t_xload(0, 3)()

            block_ids = [(b, h, qc) for b in range(B) for h in range(HL)
                         for qc in range(nqc)]
            nblk = len(block_ids)

            from collections import defaultdict
            extras = defaultdict(list)       # before the prev block's AVs
            extras_late = defaultdict(list)  # after the prev block's AVs
            tail_tasks = []
            weave = defaultdict(list)        # woven into score tasks: (pos, task)

            def sched(idx, task, late=True):
                if idx < nblk:
                    (extras_late if late else extras)[idx].append(task)
                else:
                    tail_tasks.append(task)

            # cc DMAs: slot j's data comes from block blkX's AV tasks, which
            # run as part of block blkX+1's mix -> schedule the DMA late in
            # block blkX+1 (after those AVs).
            lastA = 0
            for b in range(B):
                for q in range(4):
                    j = b * 4 + q
                    qc_of = q * 512 // qcw
                    blkA = b * nbb + qc_of
                    blkB = b * nbb + nqc + qc_of
                    sched(blkA + 1, t_ccdma(0, j))
                    sched(blkB + 1, t_ccdma(1, j))
                    lastA = max(lastA, blkA + 1)
            sched(lastA, t_a2a(0))
            sched(lastA, t_vfull(0))
            tail_tasks += [t_a2a(1), t_vfull(1)]
            if load_weights:
                sched(nbb, t_wload(wo_sb, wo))

            if nqc == 2:
                # hand schedule for qcw=1024: batch-1 prep deferred into the
                # Act-bound later blocks of batch 0 so the PE-heavy prep
                # doesn't starve the Activation engine early on.
                if pipelined_x:
                    # reload both batches' x chunks for the NEXT iteration
                    # once all of this iteration's xT readers are done
                    # (the last are batch-1 k/q/v projections in block 4)
                    for rc in range(4):
                        sched(5, t_xload(0, rc))
                        sched(6, t_xload(1, rc))
                else:
                    sched(1, t_xload(1, 0), late=False)
                    sched(1, t_xload(1, 1), late=False)
                    sched(2, t_xload(1, 2), late=False)
                    sched(2, t_xload(1, 3), late=False)
                for t in (t_kqproj(1, "k", 0), t_kqproj(1, "q", 0),
                          t_kqproj(1, "q", 1)):
                    sched(3, t, late=False)
                for st in range(NKB, NKB + 8):
                    sched(3, t_vproj(st))
                # woven into block 4 (b1,h0,qc0): k chunk c before score
                # kb 4c, remaining b1 v tiles spread along
                weave[4] = [
                    (4, t_kqproj(1, "k", 1)), (5, t_vproj(NKB + 8)),
                    (6, t_vproj(NKB + 9)), (7, t_vproj(NKB + 10)),
                    (8, t_kqproj(1, "k", 2)), (9, t_vproj(NKB + 11)),
                    (10, t_vproj(NKB + 12)), (11, t_vproj(NKB + 13)),
                    (12, t_kqproj(1, "k", 3)), (13, t_vproj(NKB + 14)),
                    (14, t_vproj(NKB + 15)),
                ]
                sched(4, t_kqproj(1, "q", 2))
                sched(4, t_kqproj(1, "q", 3))
            else:
                # generic fallback (qcw=512)
                for b in range(B):
                    for c in range(4):
                        first_qc = c * 512 // qcw
                        if b == 0 and first_qc == 0:
                            continue
                        if b == 0:
                            extras[first_qc - 1].insert(0, t_kqproj(b, "q", c))
                b1_tasks = []
                for rc in range(4):
                    b1_tasks.append(t_xload(1, rc))
                for rc in range(4):
                    b1_tasks.append(t_kqproj(1, "k", rc))
                    b1_tasks.append(t_kqproj(1, "q", rc))
                for st in range(NKB, 2 * NKB):
                    b1_tasks.append(t_vproj(st))
                nspread = nbb - 1
                for i, task in enumerate(b1_tasks):
                    sched(1 + i * nspread // len(b1_tasks), task, late=False)

            prev_av = []
            for i, (b, h, qc) in enumerate(block_ids):
                et = expp.tile([P, NKB, qcw], bf, tag="exp", name="et")
                s = score_tasks(b, h, qc, et)
                if i == 0:
                    # weave k chunks 1-3, v tiles 0..15 and (for qcw=1024)
                    # q chunks 2-3 under block 0: score kb 4c..4c+3 needs
                    # k chunk c; AV (next block) needs v tiles of batch 0;
                    # block 1 needs q chunks 2-3.  Spread so the last score
                    # tasks aren't followed by a long prep burst.
                    kp = [t_kqproj(0, "k", c) for c in (1, 2, 3)]
                    vp = [t_vproj(st) for st in range(NKB)]
                    qx = ([t_kqproj(0, "q", 2), t_kqproj(0, "q", 3)]
                          if nqc == 2 else [])
                    primary = (s[0:4] + kp[0:1] + vp[0:3]
                               + s[4:8] + kp[1:2] + vp[3:7]
                               + s[8:10] + qx[0:1] + vp[7:9]
                               + s[10:12] + kp[2:3] + vp[9:11] + qx[1:2]
                               + s[12:14] + vp[11:14]
                               + s[14:16] + vp[14:16])
                elif weave.get(i):
                    primary = []
                    wv_ = sorted(weave[i], key=lambda pt_: pt_[0])
                    wi = 0
                    for si, task in enumerate(s):
                        while wi < len(wv_) and wv_[wi][0] <= si:
                            primary.append(wv_[wi][1])
                            wi += 1
                        primary.append(task)
                    primary += [t for _, t in wv_[wi:]]
                else:
                    primary = s
                mix = extras.get(i, [])[:] + prev_av + extras_late.get(i, [])
                _interleave(primary, mix, lead=2)
                prev_av = av_tasks(b, h, qc, et)
            # final flush: the last block's cc DMAs only need the AV tasks
            # for their own row slots, so interleave them
            if nqc == 2 and len(tail_tasks) == 4:
                # prev_av is [mm0,mm1,ev0,...]; ev3 (slot 6's last row tile)
                # is at index 8
                flush = (prev_av[0:9] + tail_tasks[0:1] + prev_av[9:]
                         + tail_tasks[1:])
            else:
                flush = prev_av + tail_tasks
            for task in flush:
                task()
            if pipelined_x:
                # next iteration's first projections run while the final
                # AllToAll/vfull DMAs are in flight
                t_kqproj(0, "k", 0)()
                t_kqproj(0, "q", 0)()
                if qcw > 512:
                    t_kqproj(0, "q", 1)()


            # ---- output projection ----
            for rt in range(ROWS // P):
                for dh in range(D // 512):
                    t_oproj(rt, dh)()

        if loop_n > 1:
            t_wload(wk_sb, wk)()
            t_wload(wq_sb, wq)()
            t_wload(wv_sb, wv)()
            t_wload(wo_sb, wo)()
            if with_bias:
                t_bias_loads()()
            pipelined = (nqc == 2)
            if pipelined:
                for b in range(B):
                    for rc in range(4):
                        t_xload(b, rc)()
                t_kqproj(0, "k", 0)()
                t_kqproj(0, "q", 0)()
                if qcw > 512:
                    t_kqproj(0, "q", 1)()
            with tc.For_i(0, loop_n, 1):
                emit_body(load_weights=False, pipelined_x=pipelined)
        else:
            for rep in range(repeats):
                emit_body(load_weights=(rep == 0))

    nc.compile()
    return nc


def get_program(with_bias: bool, local_a2a: bool = False,
                repeats: int = 1, loop_n: int = 0, qcw: int = QCW, **_ignored):
    key = (with_bias, local_a2a, repeats, loop_n, qcw)
    if key not in _CACHE:
        _CACHE[key] = _build_program(with_bias, local_a2a, repeats, loop_n, qcw)
    return _CACHE[key]


def make_in_maps(x, qkv_w, qkv_b, o_w, o_b):
    """Host-side sharding: slice per-head weight rows, transpose x and the
    weights to the layouts the kernel consumes, cast to bf16."""
    bfnp = ml_dtypes.bfloat16
    x2 = np.asarray(x, np.float32).reshape(BS, D)
    # xT [128, NDC, BS]: element (p, c, r) = x[r, c*128 + p]
    xT = np.ascontiguousarray(
        x2.T.reshape(NDC, P, BS).transpose(1, 0, 2).astype(bfnp)
    )

    qkv_w = np.asarray(qkv_w, np.float32)
    o_w = np.asarray(o_w, np.float32)
    qkv_b = np.asarray(qkv_b, np.float32)
    o_b = np.asarray(o_b, np.float32)

    with_bias = bool(np.any(qkv_b) or np.any(o_b))

    woT = np.ascontiguousarray(
        o_w.T.reshape(NCORES, P, D).transpose(1, 0, 2).astype(bfnp)
    )
    ob_host = np.ascontiguousarray(o_b.reshape(1, D).astype(bfnp))

    in_maps = []
    for m in range(NCORES):
        heads = [m * HL + h for h in range(HL)]
        q_rows = np.concatenate([qkv_w[h * 3 * HD:h * 3 * HD + HD] for h in heads])
        k_rows = np.concatenate(
            [qkv_w[h * 3 * HD + HD:h * 3 * HD + 2 * HD] for h in heads]
        )
        v_rows = np.concatenate(
            [qkv_w[h * 3 * HD + 2 * HD:h * 3 * HD + 3 * HD] for h in heads]
        )

        def wt(rows):
            # [CH, D] -> [D, CH] -> [p, chunk, CH]
            return np.ascontiguousarray(
                rows.T.reshape(NDC, P, CH).transpose(1, 0, 2).astype(bfnp)
            )

        im = {
            "xT": xT,
            "wq": wt(q_rows),
            "wk": wt(k_rows),
            "wv": wt(v_rows),
            "wo": woT,
        }
        if with_bias:
            bqv = np.concatenate(
                [qkv_b[h * 3 * HD:h * 3 * HD + HD] for h in heads]
            )
            bkv = np.concatenate(
                [qkv_b[h * 3 * HD + HD:h * 3 * HD + 2 * HD] for h in heads]
            )
            bvv = np.concatenate(
                [qkv_b[h * 3 * HD + 2 * HD:h * 3 * HD + 3 * HD] for h in heads]
            )
            im["bq"] = np.ascontiguousarray(bqv.reshape(1, CH).astype(bfnp))
            im["bk"] = np.ascontiguousarray(bkv.reshape(1, CH).astype(bfnp))
            im["bv"] = np.ascontiguousarray(bvv.reshape(1, CH).astype(bfnp))
            im["ob"] = ob_host
        in_maps.append(im)
    return in_maps, with_bias


def kernel(x, qkv_w, qkv_b, o_w, o_b):
    from concourse.bass_utils import run_bass_kernel_spmd

    in_maps, with_bias = make_in_maps(x, qkv_w, qkv_b, o_w, o_b)
    nc = get_program(with_bias)
    res = run_bass_kernel_spmd(nc, in_maps, list(range(NCORES)))
    out = np.concatenate([res.results[m]["y"] for m in range(NCORES)], axis=0)
    return np.ascontiguousarray(out.reshape(B, S, D))


# revision 26
# speedup vs baseline: 1.4255x; 1.0664x over previous
"""Multi-head attention (B=2, S=2048, D=1024, H=16) on 8 TRN2 NeuronCores.

Sharding: tensor-parallel over heads (2 heads/core).  Each core computes
the qkv projection for its heads (full sequence) and attention, then an
AllToAll redistributes attention outputs so each core holds *all* heads
for a 1/8 slice of the (batch*seq) rows and runs the output projection
locally.  No cross-core reduction needed.

x is pre-transposed AND pre-cast to bf16 on the host (xT [128, D/128,
B*S]) so the device does no activation transpose at all — projections
read xT directly as the matmul moving operand.  Softmax denominators
come from a ones-column appended to V (scores are small, so exp without
max-subtraction is safe); normalization is fused into the AV eviction.

Compute dtype: bf16 matmul operands, fp32 PSUM accumulation.  Score
chunks are 1024 queries wide so each Exp activation instruction covers
[128, 1024], halving Act-engine instruction overhead vs 512-wide.
"""

import sys

sys.path.insert(0, "/opt/trn_rl_repo")

import numpy as np
import ml_dtypes

B, S, D = 2, 2048, 1024
H, HD = 16, 64
NCORES = 8
BS = B * S                 # 4096 flattened rows
HL = H // NCORES           # 2 local heads
CH = HL * HD               # 128 local q/k/v channels
ROWS = BS // NCORES        # 512 output rows per core
P = 128
NDC = D // P               # 8 chunks of the contraction dim D
NKB = S // P               # 16 key blocks per batch
QCW = 1024                 # query-chunk width (one exp instruction per kb)
HD1 = HD + 1               # value channels + ones column

_CACHE = {}


def _interleave(primary, secondary, lead=0):
    """Emit primary tasks in order, spreading secondary tasks between them.
    The first `lead` primary tasks are emitted before any secondary."""
    ns = len(secondary)
    npr = max(len(primary) - lead, 1)
    si = 0
    for i, p in enumerate(primary):
        p()
        tgt = (i + 1 - lead) * ns // npr if i >= lead else 0
        while si < tgt:
            secondary[si]()
            si += 1
    while si < ns:
        secondary[si]()
        si += 1


def _build_program(with_bias: bool, local_a2a: bool = False,
                   repeats: int = 1, loop_n: int = 0, qcw: int = QCW):
    import concourse.bass as bass
    import concourse.mybir as mybir
    import concourse.tile as tile
    from concourse import bacc
    from concourse.masks import make_identity
    from contextlib import ExitStack

    nqc = S // qcw             # query chunks per batch (2 for qcw=1024)
    nbb = HL * nqc             # blocks per batch (4)
    nqt = qcw // P             # AV row-tiles per block (8)
    dt = mybir.dt
    AF = mybir.ActivationFunctionType
    bf, f32 = dt.bfloat16, dt.float32

    nc = bacc.Bacc()

    xT_d = nc.dram_tensor("xT", [P, NDC, BS], bf, kind="ExternalInput")
    wq = nc.dram_tensor("wq", [P, NDC, CH], bf, kind="ExternalInput")
    wk = nc.dram_tensor("wk", [P, NDC, CH], bf, kind="ExternalInput")
    wv = nc.dram_tensor("wv", [P, NDC, CH], bf, kind="ExternalInput")
    wo = nc.dram_tensor("wo", [P, NCORES, D], bf, kind="ExternalInput")
    if with_bias:
        bq = nc.dram_tensor("bq", [1, CH], bf, kind="ExternalInput")
        bk = nc.dram_tensor("bk", [1, CH], bf, kind="ExternalInput")
        bv = nc.dram_tensor("bv", [1, CH], bf, kind="ExternalInput")
        ob = nc.dram_tensor("ob", [1, D], bf, kind="ExternalInput")
    y = nc.dram_tensor("y", [ROWS, D], f32, kind="ExternalOutput")

    with tile.TileContext(nc) as tc, ExitStack() as ctx:
        const = ctx.enter_context(tc.tile_pool(name="const", bufs=1))
        ident = const.tile([P, P], bf)
        make_identity(nc, ident[:])

        wq_sb = const.tile([P, NDC, CH], bf)
        wk_sb = const.tile([P, NDC, CH], bf)
        wv_sb = const.tile([P, NDC, CH], bf)
        wo_sb = const.tile([P, NCORES, D], bf)
        if with_bias:
            bq_sb = const.tile([1, CH], bf)
            bk_sb = const.tile([1, CH], bf)
            bv_sb = const.tile([1, CH], bf)
            ob_sb = const.tile([1, D], bf)
            ones_row = const.tile([1, 512], bf)

        big = ctx.enter_context(tc.tile_pool(name="big", bufs=1))
        xT = big.tile([P, NDC, BS], bf)                     # [d%128, d//128, row]
        qT = big.tile([P, BS], bf)                          # q channel-major
        kT = big.tile([P, BS], bf)                          # k channel-major
        v_aug = big.tile([P, B * NKB, HL * HD1], bf)        # v row-major + ones
        valsT = big.tile([P, BS], bf)                       # attn out, ch-major
        vfull = big.tile([P, NCORES, ROWS], bf)             # gathered, for oproj
        expp = ctx.enter_context(tc.tile_pool(name="expp", bufs=2))
        small = ctx.enter_context(tc.tile_pool(name="small", bufs=4))
        outp = ctx.enter_context(tc.tile_pool(name="outp", bufs=2))

        # PSUM budget (8 banks): pscore 2 bufs x 2 banks (1024-wide f32),
        # pbig 2 x 1 (projections), pav 1, pt 1 (AV transpose staging)
        pt = ctx.enter_context(tc.tile_pool(name="pt", bufs=1, space="PSUM"))
        pbig = ctx.enter_context(tc.tile_pool(name="pbig", bufs=2, space="PSUM"))
        pscore = ctx.enter_context(tc.tile_pool(name="pscore", bufs=2, space="PSUM"))
        pav = ctx.enter_context(tc.tile_pool(name="pav", bufs=1, space="PSUM"))

        dram = ctx.enter_context(tc.tile_pool(name="dram", bufs=1, space="DRAM"))
        # the AllToAll is split into two half-payload collectives (head 0 /
        # head 1 channel halves) so the first can run under live attention
        ccA_in = dram.tile([NCORES, HD, ROWS], bf)
        ccA_out = dram.tile([NCORES, HD, ROWS], bf)
        ccB_in = dram.tile([NCORES, HD, ROWS], bf)
        ccB_out = dram.tile([NCORES, HD, ROWS], bf)

        # ones columns for the softmax-denominator trick; value columns are
        # overwritten by the v-projection evictions
        for h in range(HL):
            nc.vector.memset(v_aug[:, :, h * HD1 + HD], 1.0)

        # ---------------- task builders ----------------

        def t_wload(wsb, wdram):
            return lambda: nc.sync.dma_start(out=wsb[:], in_=wdram[:])

        def t_bias_loads():
            def go():
                nc.sync.dma_start(out=bq_sb[:], in_=bq[:])
                nc.sync.dma_start(out=bk_sb[:], in_=bk[:])
                nc.sync.dma_start(out=bv_sb[:], in_=bv[:])
                nc.sync.dma_start(out=ob_sb[:], in_=ob[:])
                nc.vector.memset(ones_row[:], 1.0)
            return go

        def t_xload(b, rc):
            # one 512-column chunk of xT, all depth slices
            def go():
                c0 = b * S + rc * 512
                nc.sync.dma_start(
                    out=xT[:, :, c0:c0 + 512], in_=xT_d[:, :, c0:c0 + 512]
                )
            return go

        def t_vproj(st):
            def go():
                pv = pbig.tile([P, CH], f32, tag="pk", name="pv")
                for c in range(NDC):
                    nc.tensor.matmul(
                        pv[:],
                        lhsT=xT[:, c, st * P:(st + 1) * P],
                        rhs=wv_sb[:, c, :],
                        start=(c == 0),
                        stop=(c == NDC - 1 and not with_bias),
                    )
                if with_bias:
                    nc.tensor.matmul(
                        pv[:], lhsT=ones_row[:, 0:P], rhs=bv_sb[:],
                        start=False, stop=True,
                    )
                for h in range(HL):
                    nc.vector.tensor_copy(
                        out=v_aug[:, st, h * HD1:h * HD1 + HD],
                        in_=pv[:, h * HD:(h + 1) * HD],
                    )
            return go

        def t_kqproj(b, which, qc):
            # qc indexes 512-wide column chunks (0..3 per batch)
            def go():
                wsb, dst = (wk_sb, kT) if which == "k" else (wq_sb, qT)
                base = b * S + qc * 512
                pq = pbig.tile([P, 512], f32, tag="pk", name="pq")
                for c in range(NDC):
                    nc.tensor.matmul(
                        pq[:],
                        lhsT=wsb[:, c, :],
                        rhs=xT[:, c, base:base + 512],
                        start=(c == 0),
                        stop=(c == NDC - 1 and not with_bias),
                    )
                if with_bias:
                    nc.tensor.matmul(
                        pq[:],
                        lhsT=(bk_sb if which == "k" else bq_sb)[:],
                        rhs=ones_row[:],
                        start=False, stop=True,
                    )
                nc.vector.tensor_copy(out=dst[:, base:base + 512], in_=pq[:])
            return go

        # attention blocks: per (b, h, qc) -> score tasks (one per kb) and
        # AV tasks (one per 128-query tile)
        def score_tasks(b, h, qc, et):
            hp = h * HD
            qbase = b * S + qc * qcw
            tasks = []

            def mk(kb):
                def go():
                    kbase = b * S + kb * P
                    ps = pscore.tile([P, qcw], f32, tag="ps", name="ps")
                    for qh in range(qcw // 512):
                        nc.tensor.matmul(
                            ps[:, qh * 512:(qh + 1) * 512],
                            lhsT=kT[hp:hp + HD, kbase:kbase + P],
                            rhs=qT[hp:hp + HD,
                                   qbase + qh * 512:qbase + (qh + 1) * 512],
                            start=True,
                            stop=True,
                        )
                    nc.scalar.activation(et[:, kb, :], ps[:], AF.Exp, scale=0.125)
                return go

            for kb in range(NKB):
                tasks.append(mk(kb))
            return tasks

        def av_tasks(b, h, qc, et):
            """Returns interleaved [mm0, mm1, ev0, mm2, ev1, ...]: the PE
            transpose in ev_k waits on a DVE chain, so it is emitted one
            AV-tile later than its matmuls to hide the cross-engine
            latency.  4 sub-slices inside the single pav/pt banks keep the
            tiles independent."""
            hp = h * HD
            qbase = b * S + qc * qcw
            pa2 = pav.tile([P, 4, HD1], f32, tag="pa", name="pa")
            pt2 = pt.tile([P, 4, P], bf, tag="ptr", name="ptv")
            vns = [None] * nqt

            def mk_mm(qt):
                def go():
                    pa = pa2[:, qt % 4, :]
                    for kb in range(NKB):
                        nc.tensor.matmul(
                            pa,
                            lhsT=et[:, kb, qt * P:(qt + 1) * P],
                            rhs=v_aug[:, b * NKB + kb, h * HD1:(h + 1) * HD1],
                            start=(kb == 0),
                            stop=(kb == NKB - 1),
                        )
                    rc_ = small.tile([P, 1], f32, tag="rc", name="rc")
                    nc.vector.reciprocal(rc_[:], pa2[:, qt % 4, HD:HD1])
                    vn = small.tile([P, HD], bf, tag="vn", name="vn")
                    nc.vector.tensor_scalar_mul(vn[:], pa2[:, qt % 4, 0:HD], rc_[:])
                    vns[qt] = vn
                return go

            def mk_ev(qt):
                def go():
                    ptv = pt2[:, qt % 4, :]
                    nc.tensor.transpose(ptv[hp:hp + HD], vns[qt][:], ident[:])
                    col = qbase + qt * P
                    nc.vector.tensor_copy(
                        out=valsT[hp:hp + HD, col:col + P],
                        in_=pt2[hp:hp + HD, qt % 4, :],
                    )
                return go

            tasks = []
            for qt in range(nqt):
                tasks.append(mk_mm(qt))
                if qt >= 1:
                    tasks.append(mk_ev(qt - 1))
            tasks.append(mk_ev(nqt - 1))
            return tasks

        def t_ccdma(half, j):
            ccin = ccA_in if half == 0 else ccB_in
            hp = half * HD
            return lambda: nc.sync.dma_start(
                out=ccin[j], in_=valsT[hp:hp + HD, j * ROWS:(j + 1) * ROWS]
            )

        def t_a2a(half):
            ccin, ccout = (ccA_in, ccA_out) if half == 0 else (ccB_in, ccB_out)

            def go():
                if local_a2a:
                    # stand-in for the real AllToAll in loop-timing builds;
                    # p-leading layout prices it like the real collective
                    # (~3.2us vs ~3.4us measured for a 0.5MB AllToAll)
                    nc.sync.dma_start(
                        out=ccout.rearrange("i p r -> p i r"),
                        in_=ccin.rearrange("i p r -> p i r"),
                    )
                else:
                    nc.gpsimd.collective_compute(
                        "AllToAll",
                        mybir.AluOpType.bypass,
                        replica_groups=[list(range(NCORES))],
                        ins=[ccin[:]],
                        outs=[ccout[:]],
                    )
            return go

        def t_vfull(half):
            ccout = ccA_out if half == 0 else ccB_out
            hp = half * HD
            return lambda: nc.sync.dma_start(
                out=vfull[hp:hp + HD, :, :],
                in_=ccout.rearrange("i p r -> p i r"),
            )

        def t_oproj(rt, dh):
            def go():
                po = pscore.tile([P, 512], f32, tag="ps", name="po")
                for c in range(NCORES):
                    nc.tensor.matmul(
                        po[:],
                        lhsT=vfull[:, c, rt * P:(rt + 1) * P],
                        rhs=wo_sb[:, c, dh * 512:(dh + 1) * 512],
                        start=(c == 0),
                        stop=(c == NCORES - 1 and not with_bias),
                    )
                if with_bias:
                    nc.tensor.matmul(
                        po[:], lhsT=ones_row[:, 0:P],
                        rhs=ob_sb[:, dh * 512:(dh + 1) * 512],
                        start=False, stop=True,
                    )
                osb = outp.tile([P, 512], f32, tag="osb", name="osb")
                nc.vector.tensor_copy(out=osb[:], in_=po[:])
                nc.sync.dma_start(
                    out=y[rt * P:(rt + 1) * P, dh * 512:(dh + 1) * 512],
                    in_=osb[:],
                )
            return go

        # ---------------- emission (software pipeline) ----------------
        def emit_body(load_weights, pipelined_x=False):
            # front: first x chunk + k/q projections for the first block's
            # initial key blocks and query columns.  DMA queue order matters:
            # the first score matmul waits on x chunk 0 + wk + wq, so those
            # go first; wv and the remaining x chunks follow.  In pipelined
            # mode (hardware loop) the x chunks were loaded near the end of
            # the previous iteration, so the projections start immediately.
            if not pipelined_x:
                t_xload(0, 0)()
            if load_weights:
                t_wload(wk_sb, wk)()
                t_wload(wq_sb, wq)()
            if not pipelined_x:
                t_xload(0, 1)()
            if load_weights:
                t_wload(wv_sb, wv)()
                if with_bias:
                    t_bias_loads()()
            if not pipelined_x:
                t_kqproj(0, "k", 0)()
                t_kqproj(0, "q", 0)()
                if qcw > 512:
                    t_kqproj(0, "q", 1)()
                t_xload(0, 2)()
                t_xload(0, 3)()

            block_ids = [(b, h, qc) for b in range(B) for h in range(HL)
                         for qc in range(nqc)]
            nblk = len(block_ids)

            from collections import defaultdict
            extras = defaultdict(list)       # before the prev block's AVs
            extras_late = defaultdict(list)  # after the prev block's AVs
            tail_tasks = []
            weave = defaultdict(list)        # woven into score tasks: (pos, task)

            def sched(idx, task, late=True):
                if idx < nblk:
                    (extras_late if late else extras)[idx].append(task)
                else:
                    tail_tasks.append(task)

            # cc DMAs: slot j's data comes from block blkX's AV tasks, which
            # run as part of block blkX+1's mix -> schedule the DMA late in
            # block blkX+1 (after those AVs).
            lastA = 0
            for b in range(B):
                for q in range(4):
                    j = b * 4 + q
                    qc_of = q * 512 // qcw
                    blkA = b * nbb + qc_of
                    blkB = b * nbb + nqc + qc_of
                    sched(blkA + 1, t_ccdma(0, j))
                    sched(blkB + 1, t_ccdma(1, j))
                    lastA = max(lastA, blkA + 1)
            sched(lastA, t_a2a(0))
            sched(lastA, t_vfull(0))
            tail_tasks += [t_a2a(1), t_vfull(1)]
            if load_weights:
                sched(nbb, t_wload(wo_sb, wo))

            if nqc == 2:
                # hand schedule for qcw=1024: batch-1 prep deferred into the
                # Act-bound later blocks of batch 0 so the PE-heavy prep
                # doesn't starve the Activation engine early on.
                if pipelined_x:
                    # reload both batches' x chunks for the NEXT iteration
                    # once all of this iteration's xT readers are done
                    # (the last are batch-1 k/q/v projections in block 4)
                    for rc in range(4):
                        sched(5, t_xload(0, rc))
                        sched(6, t_xload(1, rc))
                else:
                    sched(1, t_xload(1, 0), late=False)
                    sched(1, t_xload(1, 1), late=False)
                    sched(2, t_xload(1, 2), late=False)
                    sched(2, t_xload(1, 3), late=False)
                for t in (t_kqproj(1, "k", 0), t_kqproj(1, "q", 0),
                          t_kqproj(1, "q", 1)):
                    sched(3, t, late=False)
                for st in range(NKB, NKB + 8):
                    sched(3, t_vproj(st))
                # woven into block 4 (b1,h0,qc0): k chunk c before score
                # kb 4c, remaining b1 v tiles spread along
                weave[4] = [
                    (4, t_kqproj(1, "k", 1)), (5, t_vproj(NKB + 8)),
                    (6, t_vproj(NKB + 9)), (7, t_vproj(NKB + 10)),
                    (8, t_kqproj(1, "k", 2)), (9, t_vproj(NKB + 11)),
                    (10, t_vproj(NKB + 12)), (11, t_vproj(NKB + 13)),
                    (12, t_kqproj(1, "k", 3)), (13, t_vproj(NKB + 14)),
                    (14, t_vproj(NKB + 15)),
                ]
                sched(4, t_kqproj(1, "q", 2))
                sched(4, t_kqproj(1, "q", 3))
            else:
                # generic fallback (qcw=512)
                for b in range(B):
                    for c in range(4):
                        first_qc = c * 512 // qcw
                        if b == 0 and first_qc == 0:
                            continue
                        if b == 0:
                            extras[first_qc - 1].insert(0, t_kqproj(b, "q", c))
                b1_tasks = []
                for rc in range(4):
                    b1_tasks.append(t_xload(1, rc))
                for rc in range(4):
                    b1_tasks.append(t_kqproj(1, "k", rc))
                    b1_tasks.append(t_kqproj(1, "q", rc))
                for st in range(NKB, 2 * NKB):
                    b1_tasks.append(t_vproj(st))
                nspread = nbb - 1
                for i, task in enumerate(b1_tasks):
                    sched(1 + i * nspread // len(b1_tasks), task, late=False)

            prev_av = []
            for i, (b, h, qc) in enumerate(block_ids):
                et = expp.tile([P, NKB, qcw], bf, tag="exp", name="et")
                s = score_tasks(b, h, qc, et)
                if i == 0:
                    # weave k chunks, v tiles and (for qcw=1024) q chunks
                    # 2-3 under block 0: score kb 4c..4c+3 needs k chunk c;
                    # AV (next block) needs v tiles of batch 0; block 1
                    # needs q chunks 2-3.  In pipelined mode k chunk 1 and
                    # v tiles 0-1 were rotated into the previous
                    # iteration's tail.
                    kp = ([] if pipelined_x else [t_kqproj(0, "k", 1)]) + \
                         [t_kqproj(0, "k", c) for c in (2, 3)]
                    vp = ([] if pipelined_x else
                          [t_vproj(st) for st in range(2)]) + \
                         [t_vproj(st) for st in range(2, NKB)]
                    qx = ([t_kqproj(0, "q", 2), t_kqproj(0, "q", 3)]
                          if nqc == 2 else [])
                    nv = len(vp)
                    primary = (s[0:4] + kp[0:len(kp) - 2] + vp[0:nv - 13]
                               + s[4:8] + kp[len(kp) - 2:len(kp) - 1]
                               + vp[nv - 13:nv - 9]
                               + s[8:10] + qx[0:1] + vp[nv - 9:nv - 7]
                               + s[10:12] + kp[len(kp) - 1:] + vp[nv - 7:nv - 5]
                               + qx[1:2]
                               + s[12:14] + vp[nv - 5:nv - 2]
                               + s[14:16] + vp[nv - 2:])
                elif weave.get(i):
                    primary = []
                    wv_ = sorted(weave[i], key=lambda pt_: pt_[0])
                    wi = 0
                    for si, task in enumerate(s):
                        while wi < len(wv_) and wv_[wi][0] <= si:
                            primary.append(wv_[wi][1])
                            wi += 1
                        primary.append(task)
                    primary += [t for _, t in wv_[wi:]]
                else:
                    primary = s
                mix = extras.get(i, [])[:] + prev_av + extras_late.get(i, [])
                _interleave(primary, mix, lead=2)
                prev_av = av_tasks(b, h, qc, et)
            # final flush: the last block's cc DMAs only need the AV tasks
            # for their own row slots, so interleave them
            if nqc == 2 and len(tail_tasks) == 4:
                # prev_av is [mm0,mm1,ev0,...]; ev3 (slot 6's last row tile)
                # is at index 8
                flush = (prev_av[0:9] + tail_tasks[0:1] + prev_av[9:]
                         + tail_tasks[1:])
            else:
                flush = prev_av + tail_tasks
            for task in flush:
                task()
            if pipelined_x:
                # next iteration's first projections run while the final
                # AllToAll/vfull DMAs are in flight
                t_kqproj(0, "k", 0)()
                t_kqproj(0, "q", 0)()
                if qcw > 512:
                    t_kqproj(0, "q", 1)()
                t_kqproj(0, "k", 1)()
                t_vproj(0)()
                t_vproj(1)()


            # ---- output projection ----
            for rt in range(ROWS // P):
                for dh in range(D // 512):
                    t_oproj(rt, dh)()

        if loop_n > 1:
            t_wload(wk_sb, wk)()
            t_wload(wq_sb, wq)()
            t_wload(wv_sb, wv)()
            t_wload(wo_sb, wo)()
            if with_bias:
                t_bias_loads()()
            pipelined = (nqc == 2)
            if pipelined:
                for b in range(B):
                    for rc in range(4):
                        t_xload(b, rc)()
                t_kqproj(0, "k", 0)()
                t_kqproj(0, "q", 0)()
                if qcw > 512:
                    t_kqproj(0, "q", 1)()
                t_kqproj(0, "k", 1)()
                t_vproj(0)()
                t_vproj(1)()
            with tc.For_i(0, loop_n, 1):
                emit_body(load_weights=False, pipelined_x=pipelined)
        else:
            for rep in range(repeats):
                emit_body(load_weights=(rep == 0))

    nc.compile()
    return nc


def get_program(with_bias: bool, local_a2a: bool = False,
                repeats: int = 1, loop_n: int = 0, qcw: int = QCW, **_ignored):
    key = (with_bias, local_a2a, repeats, loop_n, qcw)
    if key not in _CACHE:
        _CACHE[key] = _build_program(with_bias, local_a2a, repeats, loop_n, qcw)
    return _CACHE[key]


def make_in_maps(x, qkv_w, qkv_b, o_w, o_b):
    """Host-side sharding: slice per-head weight rows, transpose x and the
    weights to the layouts the kernel consumes, cast to bf16."""
    bfnp = ml_dtypes.bfloat16
    x2 = np.asarray(x, np.float32).reshape(BS, D)
    # xT [128, NDC, BS]: element (p, c, r) = x[r, c*128 + p]
    xT = np.ascontiguousarray(
        x2.T.reshape(NDC, P, BS).transpose(1, 0, 2).astype(bfnp)
    )

    qkv_w = np.asarray(qkv_w, np.float32)
    o_w = np.asarray(o_w, np.float32)
    qkv_b = np.asarray(qkv_b, np.float32)
    o_b = np.asarray(o_b, np.float32)

    with_bias = bool(np.any(qkv_b) or np.any(o_b))

    woT = np.ascontiguousarray(
        o_w.T.reshape(NCORES, P, D).transpose(1, 0, 2).astype(bfnp)
    )
    ob_host = np.ascontiguousarray(o_b.reshape(1, D).astype(bfnp))

    in_maps = []
    for m in range(NCORES):
        heads = [m * HL + h for h in range(HL)]
        q_rows = np.concatenate([qkv_w[h * 3 * HD:h * 3 * HD + HD] for h in heads])
        k_rows = np.concatenate(
            [qkv_w[h * 3 * HD + HD:h * 3 * HD + 2 * HD] for h in heads]
        )
        v_rows = np.concatenate(
            [qkv_w[h * 3 * HD + 2 * HD:h * 3 * HD + 3 * HD] for h in heads]
        )

        def wt(rows):
            # [CH, D] -> [D, CH] -> [p, chunk, CH]
            return np.ascontiguousarray(
                rows.T.reshape(NDC, P, CH).transpose(1, 0, 2).astype(bfnp)
            )

        im = {
            "xT": xT,
            "wq": wt(q_rows),
            "wk": wt(k_rows),
            "wv": wt(v_rows),
            "wo": woT,
        }
        if with_bias:
            bqv = np.concatenate(
                [qkv_b[h * 3 * HD:h * 3 * HD + HD] for h in heads]
            )
            bkv = np.concatenate(
                [qkv_b[h * 3 * HD + HD:h * 3 * HD + 2 * HD] for h in heads]
            )
            bvv = np.concatenate(
                [qkv_b[h * 3 * HD + 2 * HD:h * 3 * HD + 3 * HD] for h in heads]
            )
            im["bq"] = np.ascontiguousarray(bqv.reshape(1, CH).astype(bfnp))
            im["bk"] = np.ascontiguousarray(bkv.reshape(1, CH).astype(bfnp))
            im["bv"] = np.ascontiguousarray(bvv.reshape(1, CH).astype(bfnp))
            im["ob"] = ob_host
        in_maps.append(im)
    return in_maps, with_bias


def kernel(x, qkv_w, qkv_b, o_w, o_b):
    from concourse.bass_utils import run_bass_kernel_spmd

    in_maps, with_bias = make_in_maps(x, qkv_w, qkv_b, o_w, o_b)
    nc = get_program(with_bias)
    res = run_bass_kernel_spmd(nc, in_maps, list(range(NCORES)))
    out = np.concatenate([res.results[m]["y"] for m in range(NCORES)], axis=0)
    return np.ascontiguousarray(out.reshape(B, S, D))
